# revision 8
# baseline (speedup 1.0000x reference)
"""Trainium2 Bass kernel for nn_EncoderModel (2-layer GRU encoder).

Model: B=64, T=12, H=6400 (3H=19200 gate rows), IN0=200, 2 layers.

Sharding: tensor-parallel over the gate/output dim. Each of the 8 cores owns
PH=800 hidden units (rows [c*PH,(c+1)*PH) of each of the r/z/n gate blocks).
Each step, every core computes its PH-slice of the new hidden state and the
full h is re-assembled with an AllGather; input projections (gi) for all T
steps are computed as one batched GEMM per layer.

Per-core layouts:
  - hh GEMM: out = h @ W^T accumulated in PSUM as (batch=64 part, gate free);
    lhsT = h^T K-tiles (128,64) stationary, rhs = W^T K-tiles (128,CS) moving.
  - W^T (H, 3*PH) bf16 is streamed from HBM each step; first R K-tiles stay
    resident in SBUF.
  - h_new (64,PH) f32 -> PE-transposed to (PH,64), cast bf16 -> AllGather.
  - All biases are folded into the gi GEMM (ones row in lhsT, bias row in
    W_ih^T), except b_hh[n-gate], which joins gh_n via a K=1 ones-row matmul
    so n = tanh(gi_n + r*(h@W_hh_n^T + b_hh_n)) matches the reference.
"""

import os
from contextlib import ExitStack

import ml_dtypes
import numpy as np

import concourse.bass as bass
import concourse.mybir as mybir
import concourse.tile as tile
from concourse import bacc
from concourse.bass_utils import run_bass_kernel_spmd
from concourse.masks import make_identity

F32 = mybir.dt.float32
BF16 = mybir.dt.bfloat16
BF16NP = ml_dtypes.bfloat16
ACT = mybir.ActivationFunctionType

NCORES = 8


class Cfg:
    def __init__(self, B=64, T=12, H=6400, IN0=200, R=16, stream_bufs=4):
        self.B, self.T, self.H, self.IN0 = B, T, H, IN0
        assert B == 64
        assert H % NCORES == 0 and H % 128 == 0
        self.PH = H // NCORES            # hidden units per core
        self.G3 = 3 * self.PH            # gate rows per core
        self.KT = H // 128               # K tiles over H
        self.CS = 400 if self.PH % 400 == 0 else self.PH   # psum chunk size
        assert self.PH % self.CS == 0 and self.CS <= 512
        self.NCHG = self.PH // self.CS   # chunks per gate
        self.NCH = 3 * self.NCHG         # psum chunks per step
        assert self.NCH + 2 <= 8, "psum banks"
        self.BT = B * T
        assert self.BT % 128 == 0
        self.MT = self.BT // 128         # m tiles for gi GEMMs
        self.R = min(R, self.KT)         # resident W K-tiles
        self.stream_bufs = stream_bufs
        # K tiling for IN0+1 (ones/bias row folded in)
        k, off, self.in0_ks = IN0 + 1, 0, []
        while off < k:
            s = min(128, k - off)
            self.in0_ks.append((off, s))
            off += s
        # transpose tiles over PH
        off, self.tr = 0, []
        while off < self.PH:
            s = min(128, self.PH - off)
            self.tr.append((off, s))
            off += s


def build_nc(cfg: Cfg) -> bass.Bass:
    B, T, H, PH, G3 = cfg.B, cfg.T, cfg.H, cfg.PH, cfg.G3
    KT, CS, NCH, NCHG, MT, R = cfg.KT, cfg.CS, cfg.NCH, cfg.NCHG, cfg.MT, cfg.R
    BT = cfg.BT

    nc = bacc.Bacc("TRN2", target_bir_lowering=False, debug=False,
                   num_devices=NCORES)
    rg = [list(range(NCORES))]

    # ---- kernel I/O (per-core data via in_maps) ----
    xT = nc.dram_tensor("xT", [cfg.IN0 + 1, BT], BF16, kind="ExternalInput")
    wih0T = nc.dram_tensor("wih0T", [cfg.IN0 + 1, G3], BF16,
                           kind="ExternalInput")
    wih1T = nc.dram_tensor("wih1T", [H + 1, G3], BF16, kind="ExternalInput")
    whh0T = nc.dram_tensor("whh0T", [H, G3], BF16, kind="ExternalInput")
    whh1T = nc.dram_tensor("whh1T", [H, G3], BF16, kind="ExternalInput")
    bhhn0 = nc.dram_tensor("bhhn0", [1, G3], BF16, kind="ExternalInput")
    bhhn1 = nc.dram_tensor("bhhn1", [1, G3], BF16, kind="ExternalInput")
    out1 = nc.dram_tensor("out1", [T, PH, B], F32, kind="ExternalOutput")
    hid0 = nc.dram_tensor("hid0", [PH, B], F32, kind="ExternalOutput")
    whhT = [whh0T, whh1T]
    bhhn = [bhhn0, bhhn1]

    with tile.TileContext(nc) as tc, ExitStack() as top:
        # ---- persistent pools ----
        dram = top.enter_context(tc.tile_pool(name="dram", bufs=1,
                                              space="DRAM"))
        dram2 = top.enter_context(tc.tile_pool(name="dram2", bufs=2,
                                               space="DRAM"))
        consts = top.enter_context(tc.tile_pool(name="consts", bufs=1))
        res_pool = top.enter_context(tc.tile_pool(name="wres", bufs=1))
        stream = top.enter_context(
            tc.tile_pool(name="wstream", bufs=cfg.stream_bufs))
        ktile_pool = top.enter_context(tc.tile_pool(name="ktiles", bufs=4))
        bhh_pool = top.enter_context(tc.tile_pool(name="bhhp", bufs=1))

        # DRAM intermediates
        gi_d = [dram.tile([BT, G3], F32, name=f"gi{l}", tag=f"gi{l}")
                for l in range(2)]
        h0_all = dram.tile([T, H, B], BF16, name="h0_all", tag="h0_all")

        # constants
        ident = consts.tile([64, 64], F32, name="ident", tag="ident")
        make_identity(nc, ident[:, :])
        ones_sb = consts.tile([1, 128], BF16, name="ones_sb", tag="ones_sb")
        nc.gpsimd.memset(ones_sb[:, :], 1.0)

        def load_resident(layer):
            wres = res_pool.tile([128, max(R, 1), G3], BF16, name="wres",
                                 tag="wres")
            for ko in range(R):
                nc.sync.dma_start(wres[:, ko, :],
                                  whhT[layer][ko * 128:(ko + 1) * 128, :])
            return wres

        def gi0_gemm():
            """gi0 = [x;1]^T @ [W_ih0^T;bias] -> gi_d[0] (BT,G3) f32."""
            with ExitStack() as ctx:
                psum = ctx.enter_context(
                    tc.tile_pool(name="gi0psum", bufs=4, space="PSUM"))
                outp = ctx.enter_context(tc.tile_pool(name="gi0out", bufs=4))
                xts, wts = [], []
                for i, (off, sz) in enumerate(cfg.in0_ks):
                    xt = ktile_pool.tile([128, BT], BF16, name=f"x{i}",
                                         tag="kx")
                    nc.sync.dma_start(xt[:sz, :], xT[off:off + sz, :])
                    xts.append(xt)
                    wt = stream.tile([128, G3], BF16, name=f"w{i}", tag="wst")
                    nc.sync.dma_start(wt[:sz, :], wih0T[off:off + sz, :])
                    wts.append(wt)
                for ch in range(NCH):
                    cs = slice(ch * CS, (ch + 1) * CS)
                    for m in range(MT):
                        ms = slice(m * 128, (m + 1) * 128)
                        pt = psum.tile([128, CS], F32, name="pt", tag="gp")
                        for i, (off, sz) in enumerate(cfg.in0_ks):
                            nc.tensor.matmul(
                                pt[:, :], xts[i][:sz, ms], wts[i][:sz, cs],
                                start=(i == 0),
                                stop=(i == len(cfg.in0_ks) - 1))
                        ot = outp.tile([128, CS], F32, name="ot", tag="go")
                        nc.vector.tensor_copy(ot[:, :], pt[:, :])
                        nc.sync.dma_start(gi_d[0][ms, cs], ot[:, :])

        def gi1_gemm():
            """gi1 = [h0;1]^T @ [W_ih1^T;bias] -> gi_d[1] (BT,G3) f32."""
            with ExitStack() as ctx:
                psum = ctx.enter_context(
                    tc.tile_pool(name="gi1psum", bufs=MT, space="PSUM"))
                outp = ctx.enter_context(tc.tile_pool(name="gi1out", bufs=4))
                bias = ctx.enter_context(tc.tile_pool(name="gi1bias", bufs=1))
                bt_ = bias.tile([1, G3], BF16, name="wih1b", tag="wih1b")
                nc.sync.dma_start(bt_[:, :], wih1T[H:H + 1, :])
                for ch in range(NCH):
                    cs = slice(ch * CS, (ch + 1) * CS)
                    pts = [psum.tile([128, CS], F32, name="p1", tag="g1p")
                           for _ in range(MT)]
                    for ko in range(KT):
                        ks = slice(ko * 128, (ko + 1) * 128)
                        lt = ktile_pool.tile([128, T, B], BF16, name="lt",
                                             tag="kx")
                        nc.sync.dma_start(
                            lt[:, :, :],
                            h0_all[:, ks, :].rearrange("t k b -> k t b"))
                        wt = stream.tile([128, G3], BF16, name="w1",
                                         tag="wst")
                        nc.sync.dma_start(wt[:, :CS], wih1T[ks, cs])
                        for m in range(MT):
                            nc.tensor.matmul(
                                pts[m][:, :], lt[:, 2 * m:2 * m + 2, :],
                                wt[:, :CS], start=(ko == 0), stop=False)
                    for m in range(MT):
                        nc.tensor.matmul(     # bias row via K=1 ones matmul
                            pts[m][:, :], ones_sb[:, :128], bt_[:, cs],
                            start=False, stop=True)
                        ot = outp.tile([128, CS], F32, name="o1", tag="g1o")
                        nc.vector.tensor_copy(ot[:, :], pts[m][:, :])
                        nc.sync.dma_start(
                            gi_d[1][m * 128:(m + 1) * 128, cs], ot[:, :])

        def recurrence(layer, wres):
            """T GRU steps for one layer."""
            with ExitStack() as ctx:
                psum = ctx.enter_context(
                    tc.tile_pool(name="ghpsum", bufs=NCH, space="PSUM"))
                trps = ctx.enter_context(
                    tc.tile_pool(name="trpsum", bufs=2, space="PSUM"))
                hsb_p = ctx.enter_context(tc.tile_pool(name="hsb", bufs=2))
                gi_p = ctx.enter_context(tc.tile_pool(name="gis", bufs=2))
                ew = ctx.enter_context(tc.tile_pool(name="ew", bufs=1))
                hn_p = ctx.enter_context(tc.tile_pool(name="hnew", bufs=2))
                tr_p = ctx.enter_context(tc.tile_pool(name="htr", bufs=2))

                bh = bhh_pool.tile([1, G3], BF16, name="bh", tag="bh")
                nc.sync.dma_start(bh[:, :], bhhn[layer][:, :])

                h_prev = None
                h_gathered = None   # DRAM AP of last AllGather output
                for t in range(T):
                    # ---- gh = h_{t-1} @ W_hh^T (+ bhh_n) in PSUM ----
                    pts = [psum.tile([64, CS], F32, name="gh", tag="gh")
                           for _ in range(NCH)]
                    if t > 0:
                        hsb = hsb_p.tile([128, KT, B], BF16, name="hsbt",
                                         tag="hsbt")
                        nc.sync.dma_start(
                            hsb[:, :, :],
                            h_gathered.rearrange("(ko p) b -> p ko b", p=128))
                        for ko in range(KT):
                            if ko < R:
                                wt = wres[:, ko, :]
                            else:
                                wtile = stream.tile([128, G3], BF16,
                                                    name="ws", tag="wst")
                                nc.sync.dma_start(
                                    wtile[:, :],
                                    whhT[layer][ko * 128:(ko + 1) * 128, :])
                                wt = wtile[:, :]
                            for ch in range(NCH):
                                nc.tensor.matmul(
                                    pts[ch][:, :], hsb[:, ko, :],
                                    wt[:, ch * CS:(ch + 1) * CS],
                                    start=(ko == 0), stop=False)
                    for ch in range(NCH):
                        nc.tensor.matmul(
                            pts[ch][:, :], ones_sb[:, :64],
                            bh[:, ch * CS:(ch + 1) * CS],
                            start=(t == 0), stop=True)

                    # ---- elementwise gates ----
                    gi_t = gi_p.tile([64, G3], F32, name="git", tag="git")
                    nc.sync.dma_start(
                        gi_t[:, :], gi_d[layer][t * 64:(t + 1) * 64, :])
                    s = ew.tile([64, 2 * PH], F32, name="s", tag="s")
                    rz = ew.tile([64, 2 * PH], F32, name="rz", tag="rz")
                    npre = ew.tile([64, PH], F32, name="npre", tag="npre")
                    nadd = ew.tile([64, PH], F32, name="nadd", tag="nadd")
                    nt = ew.tile([64, PH], F32, name="nt", tag="nt")
                    d = ew.tile([64, PH], F32, name="d", tag="d")
                    e = ew.tile([64, PH], F32, name="e", tag="e")
                    h_new = hn_p.tile([64, PH], F32, name="hnw", tag="hnw")
                    for ch in range(2 * NCHG):   # r and z chunks
                        cls = slice(ch * CS, (ch + 1) * CS)
                        nc.vector.tensor_add(s[:, cls], pts[ch][:, :],
                                             gi_t[:, cls])
                    nc.scalar.activation(rz[:, :PH], s[:, :PH], ACT.Sigmoid)
                    nc.scalar.activation(rz[:, PH:], s[:, PH:], ACT.Sigmoid)
                    for j in range(NCHG):        # n chunks: r * gh_n
                        cls = slice(j * CS, (j + 1) * CS)
                        nc.vector.tensor_mul(npre[:, cls],
                                             pts[2 * NCHG + j][:, :],
                                             rz[:, cls])
                    nc.vector.tensor_add(nadd[:, :], npre[:, :],
                                         gi_t[:, 2 * PH:])
                    nc.scalar.activation(nt[:, :], nadd[:, :], ACT.Tanh)
                    if t == 0:
                        nc.vector.tensor_mul(e[:, :], rz[:, PH:], nt[:, :])
                        nc.vector.tensor_sub(h_new[:, :], nt[:, :], e[:, :])
                    else:
                        nc.vector.tensor_sub(d[:, :], h_prev[:, :], nt[:, :])
                        nc.vector.tensor_mul(e[:, :], rz[:, PH:], d[:, :])
                        nc.vector.tensor_add(h_new[:, :], nt[:, :], e[:, :])
                    h_prev = h_new

                    # ---- transpose h_new -> (PH,64): f32 out + bf16 AG ----
                    need_f32 = (layer == 1) or (t == T - 1)
                    hsh = tr_p.tile([128, len(cfg.tr), B], BF16, name="hsh",
                                    tag="hsh")
                    if need_f32:
                        htr = tr_p.tile([128, len(cfg.tr), B], F32,
                                        name="htf", tag="htf")
                    for j, (off, sz) in enumerate(cfg.tr):
                        tp = trps.tile([128, 64], F32, name="tp", tag="tp")
                        nc.tensor.transpose(tp[:sz, :],
                                            h_new[:, off:off + sz],
                                            ident[:, :])
                        if need_f32:
                            nc.vector.tensor_copy(htr[:sz, j, :], tp[:sz, :])
                        nc.vector.tensor_copy(hsh[:sz, j, :], tp[:sz, :])

                    if layer == 1:
                        for j, (off, sz) in enumerate(cfg.tr):
                            nc.sync.dma_start(out1[t, off:off + sz, :],
                                              htr[:sz, j, :])
                    if layer == 0 and t == T - 1:
                        for j, (off, sz) in enumerate(cfg.tr):
                            nc.sync.dma_start(hid0[off:off + sz, :],
                                              htr[:sz, j, :])

                    # ---- AllGather h across cores ----
                    if layer == 1 and t == T - 1:
                        continue
                    hshard = dram2.tile([PH, B], BF16, name="hshd",
                                        tag="hshd")
                    for j, (off, sz) in enumerate(cfg.tr):
                        nc.sync.dma_start(hshard[off:off + sz, :],
                                          hsh[:sz, j, :])
                    hc = dram2.tile([H, B], BF16, name="hc", tag="hc",
                                    addr_space="Shared")
                    nc.gpsimd.collective_compute(
                        "AllGather", mybir.AluOpType.bypass,
                        replica_groups=rg,
                        ins=[hshard[:, :].opt()],
                        outs=[hc[:, :].opt()],
                    )
                    if layer == 0:
                        nc.sync.dma_start(h0_all[t, :, :], hc[:, :])
                    h_gathered = hc[:, :]

        # ---- phase structure ----
        wres0 = load_resident(0)
        gi0_gemm()
        recurrence(0, wres0)
        wres1 = load_resident(1)
        gi1_gemm()
        recurrence(1, wres1)

    nc.compile()
    return nc


# --------------------------------------------------------------------------
# host side
# --------------------------------------------------------------------------

def prep_inputs(cfg: Cfg, inputs, W_ih0, W_hh0, b_ih0, b_hh0,
                W_ih1, W_hh1, b_ih1, b_hh1):
    B, T, H, PH, IN0 = cfg.B, cfg.T, cfg.H, cfg.PH, cfg.IN0
    inputs = np.asarray(inputs, np.float32)
    assert inputs.shape == (B, T, IN0)
    xT = np.empty((IN0 + 1, cfg.BT), dtype=np.float32)
    xT[:IN0] = inputs.transpose(2, 1, 0).reshape(IN0, T * B)  # col = t*B+b
    xT[IN0] = 1.0
    xT = xT.astype(BF16NP)

    arrs = {k: np.asarray(v, np.float32) for k, v in dict(
        W_ih0=W_ih0, W_hh0=W_hh0, b_ih0=b_ih0, b_hh0=b_hh0,
        W_ih1=W_ih1, W_hh1=W_hh1, b_ih1=b_ih1, b_hh1=b_hh1).items()}

    def shard(c):
        idx = np.concatenate([g * H + np.arange(c * PH, (c + 1) * PH)
                              for g in range(3)])

        def wt(W, b_i, b_h, kdim):
            out = np.empty((kdim + 1, 3 * PH), dtype=np.float32)
            out[:kdim] = W[idx].T
            bias = b_i[idx].copy()
            bias[:2 * PH] += b_h[idx][:2 * PH]   # r,z: b_ih+b_hh; n: b_ih
            out[kdim] = bias
            return out.astype(BF16NP)

        m = {
            "xT": xT,
            "wih0T": wt(arrs["W_ih0"], arrs["b_ih0"], arrs["b_hh0"], IN0),
            "wih1T": wt(arrs["W_ih1"], arrs["b_ih1"], arrs["b_hh1"], H),
            "whh0T": np.ascontiguousarray(arrs["W_hh0"][idx].T).astype(BF16NP),
            "whh1T": np.ascontiguousarray(arrs["W_hh1"][idx].T).astype(BF16NP),
        }
        for l in range(2):
            row = np.zeros((1, 3 * PH), dtype=np.float32)
            row[0, 2 * PH:] = arrs[f"b_hh{l}"][idx][2 * PH:]
            m[f"bhhn{l}"] = row.astype(BF16NP)
        return m

    return [shard(c) for c in range(NCORES)]


def assemble(cfg: Cfg, outs):
    T, B, H = cfg.T, cfg.B, cfg.H
    out1 = np.stack([outs[c]["out1"] for c in range(NCORES)])  # (8,T,PH,B)
    output = np.ascontiguousarray(
        out1.transpose(1, 3, 0, 2).reshape(T, B, H))
    hid0 = np.stack([outs[c]["hid0"] for c in range(NCORES)])  # (8,PH,B)
    h0_last = hid0.transpose(2, 0, 1).reshape(B, H)
    hidden = np.stack([h0_last, output[T - 1]], axis=0)
    return output.astype(np.float32), hidden.astype(np.float32)


LAST_RESULTS = None


def _ensure_ntff_hook():
    """The agent image's antenv lacks axon_hooks; recreate it so
    trace=True can drive NTFF profiling via the injected libaxon so."""
    try:
        from antenv.axon_hooks import get_axon_ntff_profile_hook  # noqa: F401
        return
    except ImportError:
        pass
    import sys
    import types

    import antenv

    mod = types.ModuleType("antenv.axon_hooks")
    _hook = [None]
    mod.set_axon_ntff_profile_hook = lambda h: _hook.__setitem__(0, h)
    mod.get_axon_ntff_profile_hook = lambda: _hook[0]
    sys.modules["antenv.axon_hooks"] = mod
    antenv.axon_hooks = mod
    try:
        if "/root/.axon_site" not in sys.path:
            sys.path.insert(0, "/root/.axon_site")
        from trn_agent_boot.trn_boot import _ntff_profile_via_ctypes
        so = "/opt/axon/libaxon_pjrt.so"
        if os.path.exists(so):
            mod.set_axon_ntff_profile_hook(_ntff_profile_via_ctypes(so))
    except Exception:
        pass


def kernel(inputs, W_ih0, W_hh0, b_ih0, b_hh0, W_ih1, W_hh1, b_ih1, b_hh1):
    global LAST_RESULTS
    cfg = Cfg(R=int(os.environ.get("GRU_R", "16")))
    nc = build_nc(cfg)
    in_maps = prep_inputs(cfg, inputs, W_ih0, W_hh0, b_ih0, b_hh0,
                          W_ih1, W_hh1, b_ih1, b_hh1)
    trace = bool(int(os.environ.get("GRU_TRACE", "0")))
    if trace:
        _ensure_ntff_hook()
    res = run_bass_kernel_spmd(nc, in_maps, list(range(NCORES)), trace=trace)
    LAST_RESULTS = res
    return assemble(cfg, res.results)


# revision 17
# speedup vs baseline: 1.0323x; 1.0323x over previous
"""Trainium2 Bass kernel for nn_EncoderModel (2-layer GRU encoder).

Model: B=64, T=12, H=6400 (3H=19200 gate rows), IN0=200, 2 layers.

Sharding: tensor-parallel over the gate/output dim with an interleaved
unit map: core c owns hidden units {ko*128 + c*16 + pl} (16 partition rows
of every 128-row K-tile), for each of the r/z/n gate blocks. Each step every
core computes its 800-unit slice of the new hidden state; the AllGather of
the 8 per-core (16, KT, B) shards then reconstructs the full h directly in
the packed (128 partitions, KT, B) SBUF lhsT layout — one contiguous DMA.

Per-core compute layouts:
  - hh GEMM: out = h @ W^T accumulated in PSUM as (batch=64 part, gate free);
    lhsT = h^T K-tiles (128,64) stationary, rhs = W^T K-tiles (128,CS) moving.
    PE column-tiling 2x: even K-tiles accumulate into PSUM partitions 0:64,
    odd K-tiles into 64:128 (concurrent in separate PE column groups); the
    two halves are summed by the DVE at gate-evaluation time.
  - W^T (H, 3*PH) bf16 is streamed from HBM each step; first R K-tiles stay
    resident in SBUF.
  - h_new (64,PH) f32 -> PE-transposed to (PH,64), cast bf16 -> AllGather.
  - All biases are folded into the gi GEMM (ones row in lhsT, bias row in
    W_ih^T), except b_hh[n-gate], which joins gh_n via a K=1 ones-row matmul
    so n = tanh(gi_n + r*(h@W_hh_n^T + b_hh_n)) matches the reference.
"""

import os
from contextlib import ExitStack

import ml_dtypes
import numpy as np

import concourse.bass as bass
import concourse.mybir as mybir
import concourse.tile as tile
from concourse import bacc
from concourse.bass_utils import run_bass_kernel_spmd
from concourse.masks import make_identity

F32 = mybir.dt.float32
BF16 = mybir.dt.bfloat16
BF16NP = ml_dtypes.bfloat16
ACT = mybir.ActivationFunctionType

NCORES = 8
PL = 128 // NCORES   # partition rows per core within each K-tile


class Cfg:
    def __init__(self, B=64, T=12, H=6400, IN0=200, R=16, stream_bufs=5):
        self.B, self.T, self.H, self.IN0 = B, T, H, IN0
        assert B == 64
        assert H % 128 == 0
        self.PH = H // NCORES            # hidden units per core
        self.G3 = 3 * self.PH            # gate rows per core
        self.KT = H // 128               # K tiles over H
        assert self.PH == self.KT * PL
        self.CS = 400 if self.PH % 400 == 0 else self.PH   # psum chunk size
        assert self.PH % self.CS == 0 and self.CS <= 512
        self.NCHG = self.PH // self.CS   # chunks per gate
        self.NCH = 3 * self.NCHG         # psum chunks per step
        assert self.NCH + 2 <= 8, "psum banks"
        self.BT = B * T
        assert self.BT % 128 == 0
        self.MT = self.BT // 128         # m tiles for gi GEMMs
        self.R = min(R, self.KT)         # resident W K-tiles
        self.stream_bufs = stream_bufs
        # K tiling for IN0+1 (ones/bias row folded in)
        k, off, self.in0_ks = IN0 + 1, 0, []
        while off < k:
            s = min(128, k - off)
            self.in0_ks.append((off, s))
            off += s
        # transpose tiles over PH (each tile covers sz//PL K-tiles)
        off, self.tr = 0, []
        while off < self.PH:
            s = min(128, self.PH - off)
            assert s % PL == 0
            self.tr.append((off, s))
            off += s


def build_nc(cfg: Cfg) -> bass.Bass:
    B, T, H, PH, G3 = cfg.B, cfg.T, cfg.H, cfg.PH, cfg.G3
    KT, CS, NCH, NCHG, MT, R = cfg.KT, cfg.CS, cfg.NCH, cfg.NCHG, cfg.MT, cfg.R
    BT = cfg.BT
    coltile = KT >= 2
    last_odd = KT - 1 if (KT - 1) % 2 == 1 else KT - 2

    nc = bacc.Bacc("TRN2", target_bir_lowering=False, debug=False,
                   num_devices=NCORES)
    rg = [list(range(NCORES))]

    # ---- kernel I/O (per-core data via in_maps) ----
    xT = nc.dram_tensor("xT", [cfg.IN0 + 1, BT], BF16, kind="ExternalInput")
    wih0T = nc.dram_tensor("wih0T", [cfg.IN0 + 1, G3], BF16,
                           kind="ExternalInput")
    wih1T = nc.dram_tensor("wih1T", [H + 1, G3], BF16, kind="ExternalInput")
    whh0T = nc.dram_tensor("whh0T", [H, G3], BF16, kind="ExternalInput")
    whh1T = nc.dram_tensor("whh1T", [H, G3], BF16, kind="ExternalInput")
    bhhn0 = nc.dram_tensor("bhhn0", [1, G3], BF16, kind="ExternalInput")
    bhhn1 = nc.dram_tensor("bhhn1", [1, G3], BF16, kind="ExternalInput")
    out1 = nc.dram_tensor("out1", [T, PH, B], F32, kind="ExternalOutput")
    hid0 = nc.dram_tensor("hid0", [PH, B], F32, kind="ExternalOutput")
    whhT = [whh0T, whh1T]
    bhhn = [bhhn0, bhhn1]

    with tile.TileContext(nc) as tc, ExitStack() as top:
        # ---- persistent pools ----
        dram = top.enter_context(tc.tile_pool(name="dram", bufs=1,
                                              space="DRAM"))
        dram2 = top.enter_context(tc.tile_pool(name="dram2", bufs=2,
                                               space="DRAM"))
        consts = top.enter_context(tc.tile_pool(name="consts", bufs=1))
        res_pool = top.enter_context(tc.tile_pool(name="wres", bufs=1))
        stream = top.enter_context(
            tc.tile_pool(name="wstream", bufs=cfg.stream_bufs))
        ktile_pool = top.enter_context(tc.tile_pool(name="ktiles", bufs=3))
        bhh_pool = top.enter_context(tc.tile_pool(name="bhhp", bufs=1))

        # DRAM intermediates. h0_all is laid out (KT, 128, T, B) so the gi1
        # GEMM reads one contiguous (128, T*B) lhsT tile per K-tile.
        gi_d = [dram.tile([BT, G3], BF16, name=f"gi{l}", tag=f"gi{l}")
                for l in range(2)]
        h0_all = dram.tile([KT, 128, T, B], BF16, name="h0_all", tag="h0a")

        # constants
        ident = consts.tile([64, 64], F32, name="ident", tag="ident")
        make_identity(nc, ident[:, :])
        ones_sb = consts.tile([1, 128], BF16, name="ones_sb", tag="ones_sb")
        nc.gpsimd.memset(ones_sb[:, :], 1.0)
        zrow = consts.tile([1, CS], BF16, name="zrow", tag="zrow")
        nc.gpsimd.memset(zrow[:, :], 0.0)

        def load_resident(layer):
            wres = res_pool.tile([128, max(R, 1), G3], BF16, name="wres",
                                 tag="wres")
            for ko in range(R):
                nc.sync.dma_start(wres[:, ko, :],
                                  whhT[layer][ko * 128:(ko + 1) * 128, :])
            return wres

        def gi0_gemm():
            """gi0 = [x;1]^T @ [W_ih0^T;bias] -> gi_d[0] (BT,G3) bf16."""
            with ExitStack() as ctx:
                psum = ctx.enter_context(
                    tc.tile_pool(name="gi0psum", bufs=4, space="PSUM"))
                outp = ctx.enter_context(tc.tile_pool(name="gi0out", bufs=4))
                xts, wts = [], []
                for i, (off, sz) in enumerate(cfg.in0_ks):
                    xt = ktile_pool.tile([128, BT], BF16, name=f"x{i}",
                                         tag="kx")
                    nc.sync.dma_start(xt[:sz, :], xT[off:off + sz, :])
                    xts.append(xt)
                    wt = stream.tile([128, G3], BF16, name=f"w{i}", tag="wst")
                    nc.sync.dma_start(wt[:sz, :], wih0T[off:off + sz, :])
                    wts.append(wt)
                for ch in range(NCH):
                    cs = slice(ch * CS, (ch + 1) * CS)
                    for m in range(MT):
                        ms = slice(m * 128, (m + 1) * 128)
                        pt = psum.tile([128, CS], F32, name="pt", tag="gp")
                        for i, (off, sz) in enumerate(cfg.in0_ks):
                            nc.tensor.matmul(
                                pt[:, :], xts[i][:sz, ms], wts[i][:sz, cs],
                                start=(i == 0),
                                stop=(i == len(cfg.in0_ks) - 1))
                        ot = outp.tile([128, CS], BF16, name="ot", tag="go")
                        nc.vector.tensor_copy(ot[:, :], pt[:, :])
                        nc.sync.dma_start(gi_d[0][ms, cs], ot[:, :])

        def gi1_gemm():
            """gi1 = [h0;1]^T @ [W_ih1^T;bias] -> gi_d[1] (BT,G3) bf16."""
            with ExitStack() as ctx:
                psum = ctx.enter_context(
                    tc.tile_pool(name="gi1psum", bufs=MT, space="PSUM"))
                outp = ctx.enter_context(tc.tile_pool(name="gi1out", bufs=4))
                bias = ctx.enter_context(tc.tile_pool(name="gi1bias", bufs=1))
                bt_ = bias.tile([1, G3], BF16, name="wih1b", tag="wih1b")
                nc.sync.dma_start(bt_[:, :], wih1T[H:H + 1, :])
                for ch in range(NCH):
                    cs = slice(ch * CS, (ch + 1) * CS)
                    pts = [psum.tile([128, CS], F32, name="p1", tag="g1p")
                           for _ in range(MT)]
                    for ko in range(KT):
                        ks = slice(ko * 128, (ko + 1) * 128)
                        lt = ktile_pool.tile([128, T, B], BF16, name="lt",
                                             tag="kx")
                        nc.sync.dma_start(lt[:, :, :], h0_all[ko, :, :, :])
                        wt = stream.tile([128, G3], BF16, name="w1",
                                         tag="wst")
                        nc.sync.dma_start(wt[:, :CS], wih1T[ks, cs])
                        for m in range(MT):
                            nc.tensor.matmul(
                                pts[m][:, :], lt[:, 2 * m:2 * m + 2, :],
                                wt[:, :CS], start=(ko == 0), stop=False)
                    for m in range(MT):
                        nc.tensor.matmul(     # bias row via K=1 ones matmul
                            pts[m][:, :], ones_sb[:, :128], bt_[:, cs],
                            start=False, stop=True)
                        ot = outp.tile([128, CS], BF16, name="o1", tag="g1o")
                        nc.vector.tensor_copy(ot[:, :], pts[m][:, :])
                        nc.sync.dma_start(
                            gi_d[1][m * 128:(m + 1) * 128, cs], ot[:, :])

        def recurrence(layer, wres):
            """T GRU steps for one layer."""
            with ExitStack() as ctx:
                psum = ctx.enter_context(
                    tc.tile_pool(name="ghpsum", bufs=NCH, space="PSUM"))
                trps = ctx.enter_context(
                    tc.tile_pool(name="trpsum", bufs=2, space="PSUM"))
                hsb_p = ctx.enter_context(tc.tile_pool(name="hsb", bufs=2))
                gi_p = ctx.enter_context(tc.tile_pool(name="gis", bufs=2))
                ew = ctx.enter_context(tc.tile_pool(name="ew", bufs=1))
                hn_p = ctx.enter_context(tc.tile_pool(name="hnew", bufs=2))
                tr_p = ctx.enter_context(tc.tile_pool(name="htr", bufs=2))

                bh = bhh_pool.tile([1, G3], BF16, name="bh", tag="bh")
                nc.sync.dma_start(bh[:, :], bhhn[layer][:, :])

                h_prev = None
                h_gathered = None   # DRAM AP of last AllGather output
                # chunk ch lives in PSUM tile ch//2, partition half ch%2 —
                # pairs of chunks share a bank and run in separate PE column
                # groups concurrently.
                nbank = (NCH + 1) // 2 if coltile else NCH

                def chunk_ap(pts, ch):
                    if coltile:
                        return pts[ch // 2][(ch % 2) * 64:(ch % 2) * 64 + 64,
                                            :]
                    return pts[ch][0:64, :]

                for t in range(T):
                    # ---- gh = h_{t-1} @ W_hh^T (+ bhh_n) in PSUM ----
                    pts = [psum.tile([128, CS] if coltile else [64, CS], F32,
                                     name="gh", tag="gh")
                           for _ in range(nbank)]
                    # Full-bank zero "seed" starts each bank's accumulation
                    # group: it spans both partition halves, so every later
                    # matmul overlaps it (Tile orders them after it) and
                    # accumulates onto zero via has_written set by the seed.
                    np_ = 128 if coltile else 64
                    for bk in range(nbank):
                        nc.tensor.matmul(pts[bk][:np_, :],
                                         ones_sb[:, :np_], zrow[:, :],
                                         start=True, stop=False)
                    if t > 0:
                        hsb = hsb_p.tile([128, KT, B], BF16, name="hsbt",
                                         tag="hsbt")
                        nc.sync.dma_start(hsb[:, :, :], h_gathered)
                        for ko in range(KT):
                            if ko < R:
                                wt = wres[:, ko, :]
                            else:
                                wtile = stream.tile([128, G3], BF16,
                                                    name="ws", tag="wst")
                                nc.sync.dma_start(
                                    wtile[:, :],
                                    whhT[layer][ko * 128:(ko + 1) * 128, :])
                                wt = wtile[:, :]
                            for ch in range(NCH):
                                nc.tensor.matmul(
                                    chunk_ap(pts, ch), hsb[:, ko, :],
                                    wt[:, ch * CS:(ch + 1) * CS],
                                    start=False, stop=False)
                    for ch in range(NCH):
                        nc.tensor.matmul(
                            chunk_ap(pts, ch), ones_sb[:, :64],
                            bh[:, ch * CS:(ch + 1) * CS],
                            start=False, stop=False)
                    for bk in range(nbank):   # close each bank's group
                        nc.tensor.matmul(pts[bk][:np_, :],
                                         ones_sb[:, :np_], zrow[:, :],
                                         start=False, stop=True)

                    # ---- elementwise gates ----
                    gi_t = gi_p.tile([64, G3], BF16, name="git", tag="git")
                    nc.sync.dma_start(
                        gi_t[:, :], gi_d[layer][t * 64:(t + 1) * 64, :])
                    s = ew.tile([64, 2 * PH], F32, name="s", tag="s")
                    rz = ew.tile([64, 2 * PH], F32, name="rz", tag="rz")
                    npre = ew.tile([64, PH], F32, name="npre", tag="npre")
                    nadd = ew.tile([64, PH], F32, name="nadd", tag="nadd")
                    nt = ew.tile([64, PH], F32, name="nt", tag="nt")
                    d = ew.tile([64, PH], F32, name="d", tag="d")
                    e = ew.tile([64, PH], F32, name="e", tag="e")
                    h_new = hn_p.tile([64, PH], F32, name="hnw", tag="hnw")
                    for ch in range(2 * NCHG):   # r and z chunks
                        cls = slice(ch * CS, (ch + 1) * CS)
                        nc.vector.tensor_add(s[:, cls], chunk_ap(pts, ch),
                                             gi_t[:, cls])
                    nc.scalar.activation(rz[:, :PH], s[:, :PH], ACT.Sigmoid)
                    nc.scalar.activation(rz[:, PH:], s[:, PH:], ACT.Sigmoid)
                    for j in range(NCHG):        # n chunks: r * gh_n
                        cls = slice(j * CS, (j + 1) * CS)
                        nc.vector.tensor_mul(npre[:, cls],
                                             chunk_ap(pts, 2 * NCHG + j),
                                             rz[:, cls])
                    nc.vector.tensor_add(nadd[:, :], npre[:, :],
                                         gi_t[:, 2 * PH:])
                    nc.scalar.activation(nt[:, :], nadd[:, :], ACT.Tanh)
                    if t == 0:
                        nc.vector.tensor_mul(e[:, :], rz[:, PH:], nt[:, :])
                        nc.vector.tensor_sub(h_new[:, :], nt[:, :], e[:, :])
                    else:
                        nc.vector.tensor_sub(d[:, :], h_prev[:, :], nt[:, :])
                        nc.vector.tensor_mul(e[:, :], rz[:, PH:], d[:, :])
                        nc.vector.tensor_add(h_new[:, :], nt[:, :], e[:, :])
                    h_prev = h_new

                    # ---- transpose h_new -> (PH,64): f32 out + bf16 AG ----
                    need_f32 = (layer == 1) or (t == T - 1)
                    hsh = tr_p.tile([128, len(cfg.tr), B], BF16, name="hsh",
                                    tag="hsh")
                    if need_f32:
                        htr = tr_p.tile([128, len(cfg.tr), B], F32,
                                        name="htf", tag="htf")
                    for j, (off, sz) in enumerate(cfg.tr):
                        tp = trps.tile([128, 64], F32, name="tp", tag="tp")
                        nc.tensor.transpose(tp[:sz, :],
                                            h_new[:, off:off + sz],
                                            ident[:, :])
                        if need_f32:
                            nc.vector.tensor_copy(htr[:sz, j, :], tp[:sz, :])
                        nc.vector.tensor_copy(hsh[:sz, j, :], tp[:sz, :])

                    if layer == 1:
                        for j, (off, sz) in enumerate(cfg.tr):
                            nc.sync.dma_start(out1[t, off:off + sz, :],
                                              htr[:sz, j, :])
                    if layer == 0 and t == T - 1:
                        for j, (off, sz) in enumerate(cfg.tr):
                            nc.sync.dma_start(hid0[off:off + sz, :],
                                              htr[:sz, j, :])

                    # ---- AllGather h across cores ----
                    if layer == 1 and t == T - 1:
                        continue
                    # local shard (PL, KT, B): [pl, ko, b] = h_new^T[ko*PL+pl]
                    hshard = dram2.tile([PL, KT, B], BF16, name="hshd",
                                        tag="hshd")
                    for j, (off, sz) in enumerate(cfg.tr):
                        for kl in range(sz // PL):
                            nc.sync.dma_start(
                                hshard[:, off // PL + kl, :],
                                hsh[kl * PL:(kl + 1) * PL, j, :])
                    hc = dram2.tile([128, KT, B], BF16, name="hc", tag="hc",
                                    addr_space="Shared")
                    nc.gpsimd.collective_compute(
                        "AllGather", mybir.AluOpType.bypass,
                        replica_groups=rg,
                        ins=[hshard[:, :, :].opt()],
                        outs=[hc[:, :, :].opt()],
                    )
                    if layer == 0:
                        nc.sync.dma_start(
                            h0_all[:, :, t, :].rearrange("ko p b -> p ko b"),
                            hc[:, :, :])
                    h_gathered = hc[:, :, :]

        # ---- phase structure ----
        gi0_gemm()
        wres0 = load_resident(0)
        recurrence(0, wres0)
        wres1 = load_resident(1)
        gi1_gemm()
        recurrence(1, wres1)

    nc.compile()
    return nc


# --------------------------------------------------------------------------
# host side
# --------------------------------------------------------------------------

def _unit_order(cfg: Cfg, c: int) -> np.ndarray:
    """Global hidden-unit index for core c's local units 0..PH-1."""
    ar = np.arange(cfg.PH)
    return (ar // PL) * 128 + c * PL + (ar % PL)


def prep_inputs(cfg: Cfg, inputs, W_ih0, W_hh0, b_ih0, b_hh0,
                W_ih1, W_hh1, b_ih1, b_hh1):
    B, T, H, PH, IN0 = cfg.B, cfg.T, cfg.H, cfg.PH, cfg.IN0
    inputs = np.asarray(inputs, np.float32)
    assert inputs.shape == (B, T, IN0)
    xT = np.empty((IN0 + 1, cfg.BT), dtype=np.float32)
    xT[:IN0] = inputs.transpose(2, 1, 0).reshape(IN0, T * B)  # col = t*B+b
    xT[IN0] = 1.0
    xT = xT.astype(BF16NP)

    arrs = {k: np.asarray(v, np.float32) for k, v in dict(
        W_ih0=W_ih0, W_hh0=W_hh0, b_ih0=b_ih0, b_hh0=b_hh0,
        W_ih1=W_ih1, W_hh1=W_hh1, b_ih1=b_ih1, b_hh1=b_hh1).items()}

    def shard(c):
        unit = _unit_order(cfg, c)
        idx = np.concatenate([g * H + unit for g in range(3)])

        def wt(W, b_i, b_h, kdim):
            out = np.empty((kdim + 1, 3 * PH), dtype=np.float32)
            out[:kdim] = W[idx].T
            bias = b_i[idx].copy()
            bias[:2 * PH] += b_h[idx][:2 * PH]   # r,z: b_ih+b_hh; n: b_ih
            out[kdim] = bias
            return out.astype(BF16NP)

        m = {
            "xT": xT,
            "wih0T": wt(arrs["W_ih0"], arrs["b_ih0"], arrs["b_hh0"], IN0),
            "wih1T": wt(arrs["W_ih1"], arrs["b_ih1"], arrs["b_hh1"], H),
            "whh0T": np.ascontiguousarray(arrs["W_hh0"][idx].T).astype(BF16NP),
            "whh1T": np.ascontiguousarray(arrs["W_hh1"][idx].T).astype(BF16NP),
        }
        for l in range(2):
            row = np.zeros((1, 3 * PH), dtype=np.float32)
            row[0, 2 * PH:] = arrs[f"b_hh{l}"][idx][2 * PH:]
            m[f"bhhn{l}"] = row.astype(BF16NP)
        return m

    return [shard(c) for c in range(NCORES)]


def assemble(cfg: Cfg, outs):
    T, B, H, KT = cfg.T, cfg.B, cfg.H, cfg.KT
    # local unit j=(ko*PL+pl) on core c -> global unit ko*128 + c*PL + pl
    out1 = np.stack([outs[c]["out1"] for c in range(NCORES)])  # (8,T,PH,B)
    out1 = out1.reshape(NCORES, T, KT, PL, B)
    output = np.ascontiguousarray(
        out1.transpose(1, 4, 2, 0, 3).reshape(T, B, H))
    hid0 = np.stack([outs[c]["hid0"] for c in range(NCORES)])  # (8,PH,B)
    hid0 = hid0.reshape(NCORES, KT, PL, B)
    h0_last = hid0.transpose(3, 1, 0, 2).reshape(B, H)
    hidden = np.stack([h0_last, output[T - 1]], axis=0)
    return output.astype(np.float32), hidden.astype(np.float32)


LAST_RESULTS = None


def _ensure_ntff_hook():
    """The agent image's antenv lacks axon_hooks; recreate it so
    trace=True can drive NTFF profiling via the injected libaxon so."""
    try:
        from antenv.axon_hooks import get_axon_ntff_profile_hook  # noqa: F401
        return
    except ImportError:
        pass
    import sys
    import types

    import antenv

    mod = types.ModuleType("antenv.axon_hooks")
    _hook = [None]
    mod.set_axon_ntff_profile_hook = lambda h: _hook.__setitem__(0, h)
    mod.get_axon_ntff_profile_hook = lambda: _hook[0]
    sys.modules["antenv.axon_hooks"] = mod
    antenv.axon_hooks = mod
    try:
        if "/root/.axon_site" not in sys.path:
            sys.path.insert(0, "/root/.axon_site")
        from trn_agent_boot.trn_boot import _ntff_profile_via_ctypes
        so = "/opt/axon/libaxon_pjrt.so"
        if os.path.exists(so):
            mod.set_axon_ntff_profile_hook(_ntff_profile_via_ctypes(so))
    except Exception:
        pass


def kernel(inputs, W_ih0, W_hh0, b_ih0, b_hh0, W_ih1, W_hh1, b_ih1, b_hh1):
    global LAST_RESULTS
    cfg = Cfg(R=int(os.environ.get("GRU_R", "16")),
              stream_bufs=int(os.environ.get("GRU_SB", "5")))
    nc = build_nc(cfg)
    in_maps = prep_inputs(cfg, inputs, W_ih0, W_hh0, b_ih0, b_hh0,
                          W_ih1, W_hh1, b_ih1, b_hh1)
    trace = bool(int(os.environ.get("GRU_TRACE", "0")))
    if trace:
        _ensure_ntff_hook()
    res = run_bass_kernel_spmd(nc, in_maps, list(range(NCORES)), trace=trace)
    LAST_RESULTS = res
    return assemble(cfg, res.results)


# revision 29
# speedup vs baseline: 1.2950x; 1.2545x over previous
"""Trainium2 Bass kernel for nn_EncoderModel (2-layer GRU encoder).

Model: B=64, T=12, H=6400 (3H=19200 gate rows), IN0=200, 2 layers.

Sharding: tensor-parallel over the gate/output dim with an interleaved
unit map: core c owns hidden units {ko*128 + c*16 + pl} (16 partition rows
of every 128-row K-tile), for each of the r/z/n gate blocks. Each step every
core computes its 800-unit slice of the new hidden state; the AllGather of
the 8 per-core (16, KT, B) shards then reconstructs the full h directly in
the packed (128 partitions, KT, B) SBUF lhsT layout — one contiguous DMA.

Per-core compute layouts:
  - hh GEMM: out = h @ W^T accumulated in PSUM as (batch=64 part, gate free);
    lhsT = h^T K-tiles (128,64) stationary, rhs = W^T K-tiles (128,CS) moving.
    PE column-tiling 2x: even K-tiles accumulate into PSUM partitions 0:64,
    odd K-tiles into 64:128 (concurrent in separate PE column groups); the
    two halves are summed by the DVE at gate-evaluation time.
  - W^T (H, 3*PH) bf16 is streamed from HBM each step; first R K-tiles stay
    resident in SBUF.
  - h_new (64,PH) f32 -> PE-transposed to (PH,64), cast bf16 -> AllGather.
  - All biases are folded into the gi GEMM (ones row in lhsT, bias row in
    W_ih^T), except b_hh[n-gate], which joins gh_n via a K=1 ones-row matmul
    so n = tanh(gi_n + r*(h@W_hh_n^T + b_hh_n)) matches the reference.
"""

import os
from contextlib import ExitStack

import ml_dtypes
import numpy as np

import concourse.bass as bass
import concourse.mybir as mybir
import concourse.tile as tile
from concourse import bacc
from concourse.bass_utils import run_bass_kernel_spmd
from concourse.masks import make_identity

F32 = mybir.dt.float32
BF16 = mybir.dt.bfloat16
BF16NP = ml_dtypes.bfloat16
ACT = mybir.ActivationFunctionType

NCORES = 8
PL = 128 // NCORES   # partition rows per core within each K-tile


class Cfg:
    def __init__(self, B=64, T=12, H=6400, IN0=200, R=16, stream_bufs=5):
        self.B, self.T, self.H, self.IN0 = B, T, H, IN0
        assert B == 64
        assert H % 128 == 0
        self.PH = H // NCORES            # hidden units per core
        self.G3 = 3 * self.PH            # gate rows per core
        self.KT = H // 128               # K tiles over H
        assert self.PH == self.KT * PL
        self.CS = 400 if self.PH % 400 == 0 else self.PH   # psum chunk size
        assert self.PH % self.CS == 0 and self.CS <= 512
        self.NCHG = self.PH // self.CS   # chunks per gate
        self.NCH = 3 * self.NCHG         # psum chunks per step
        assert self.NCH + 2 <= 8, "psum banks"
        self.BT = B * T
        assert self.BT % 128 == 0
        self.MT = self.BT // 128         # m tiles for gi GEMMs
        self.R = min(R, self.KT)         # resident W K-tiles
        self.stream_bufs = stream_bufs
        # K tiling for IN0+1 (ones/bias row folded in)
        k, off, self.in0_ks = IN0 + 1, 0, []
        while off < k:
            s = min(128, k - off)
            self.in0_ks.append((off, s))
            off += s
        # transpose tiles over PH (each tile covers sz//PL K-tiles)
        off, self.tr = 0, []
        while off < self.PH:
            s = min(128, self.PH - off)
            assert s % PL == 0
            self.tr.append((off, s))
            off += s


def build_nc(cfg: Cfg) -> bass.Bass:
    B, T, H, PH, G3 = cfg.B, cfg.T, cfg.H, cfg.PH, cfg.G3
    KT, CS, NCH, NCHG, MT, R = cfg.KT, cfg.CS, cfg.NCH, cfg.NCHG, cfg.MT, cfg.R
    BT = cfg.BT
    coltile = KT >= 2
    last_odd = KT - 1 if (KT - 1) % 2 == 1 else KT - 2

    nc = bacc.Bacc("TRN2", target_bir_lowering=False, debug=False,
                   num_devices=NCORES)
    rg = [list(range(NCORES))]

    # ---- kernel I/O (per-core data via in_maps) ----
    xT = nc.dram_tensor("xT", [cfg.IN0 + 1, BT], BF16, kind="ExternalInput")
    wih0T = nc.dram_tensor("wih0T", [cfg.IN0 + 1, G3], BF16,
                           kind="ExternalInput")
    wih1T = nc.dram_tensor("wih1T", [H + 1, G3], BF16, kind="ExternalInput")
    whh0T = nc.dram_tensor("whh0T", [H, G3], BF16, kind="ExternalInput")
    whh1T = nc.dram_tensor("whh1T", [H, G3], BF16, kind="ExternalInput")
    bhhn0 = nc.dram_tensor("bhhn0", [1, G3], BF16, kind="ExternalInput")
    bhhn1 = nc.dram_tensor("bhhn1", [1, G3], BF16, kind="ExternalInput")
    out1 = nc.dram_tensor("out1", [T, PH, B], F32, kind="ExternalOutput")
    hid0 = nc.dram_tensor("hid0", [PH, B], F32, kind="ExternalOutput")
    whhT = [whh0T, whh1T]
    bhhn = [bhhn0, bhhn1]

    with tile.TileContext(nc) as tc, ExitStack() as top:
        # ---- persistent pools ----
        dram = top.enter_context(tc.tile_pool(name="dram", bufs=1,
                                              space="DRAM"))
        dram2 = top.enter_context(tc.tile_pool(name="dram2", bufs=2,
                                               space="DRAM"))
        consts = top.enter_context(tc.tile_pool(name="consts", bufs=1))
        res_pool = top.enter_context(tc.tile_pool(name="wres", bufs=1))
        stream = top.enter_context(
            tc.tile_pool(name="wstream", bufs=cfg.stream_bufs))
        ktile_pool = top.enter_context(tc.tile_pool(name="ktiles", bufs=3))
        bhh_pool = top.enter_context(tc.tile_pool(name="bhhp", bufs=1))

        # DRAM intermediates
        gi_d = [dram.tile([BT, G3], BF16, name=f"gi{l}", tag=f"gi{l}")
                for l in range(2)]

        # constants
        ident = consts.tile([64, 64], F32, name="ident", tag="ident")
        make_identity(nc, ident[:, :])
        ones_sb = consts.tile([1, 128], BF16, name="ones_sb", tag="ones_sb")
        nc.gpsimd.memset(ones_sb[:, :], 1.0)
        zrow = consts.tile([1, CS], BF16, name="zrow", tag="zrow")
        nc.gpsimd.memset(zrow[:, :], 0.0)

        def load_resident(layer):
            wres = res_pool.tile([128, max(R, 1), G3], BF16, name="wres",
                                 tag="wres")
            for ko in range(R):
                nc.sync.dma_start(wres[:, ko, :],
                                  whhT[layer][ko * 128:(ko + 1) * 128, :])
            return wres

        def gi0_gemm():
            """gi0 = [x;1]^T @ [W_ih0^T;bias] -> gi_d[0] (BT,G3) bf16."""
            with ExitStack() as ctx:
                psum = ctx.enter_context(
                    tc.tile_pool(name="gi0psum", bufs=4, space="PSUM"))
                outp = ctx.enter_context(tc.tile_pool(name="gi0out", bufs=4))
                xts, wts = [], []
                for i, (off, sz) in enumerate(cfg.in0_ks):
                    xt = ktile_pool.tile([128, BT], BF16, name=f"x{i}",
                                         tag="kx")
                    nc.sync.dma_start(xt[:sz, :], xT[off:off + sz, :])
                    xts.append(xt)
                    wt = stream.tile([128, G3], BF16, name=f"w{i}", tag="wst")
                    nc.sync.dma_start(wt[:sz, :], wih0T[off:off + sz, :])
                    wts.append(wt)
                for ch in range(NCH):
                    cs = slice(ch * CS, (ch + 1) * CS)
                    for m in range(MT):
                        ms = slice(m * 128, (m + 1) * 128)
                        pt = psum.tile([128, CS], F32, name="pt", tag="gp")
                        for i, (off, sz) in enumerate(cfg.in0_ks):
                            nc.tensor.matmul(
                                pt[:, :], xts[i][:sz, ms], wts[i][:sz, cs],
                                start=(i == 0),
                                stop=(i == len(cfg.in0_ks) - 1))
                        ot = outp.tile([128, CS], BF16, name="ot", tag="go")
                        nc.vector.tensor_copy(ot[:, :], pt[:, :])
                        nc.sync.dma_start(gi_d[0][ms, cs], ot[:, :])

        def gi1_gemm(h0_tiles):
            """gi1 = [h0;1]^T @ [W_ih1^T;bias] -> gi_d[1] (BT,G3) bf16.

            h0_tiles: layer-0's T AllGather outputs, each (128, KT, B).
            All of h0 is preloaded into SBUF once (T contiguous DMAs)."""
            with ExitStack() as ctx:
                psum = ctx.enter_context(
                    tc.tile_pool(name="gi1psum", bufs=MT, space="PSUM"))
                outp = ctx.enter_context(tc.tile_pool(name="gi1out", bufs=4))
                bias = ctx.enter_context(tc.tile_pool(name="gi1bias", bufs=1))
                h0p = ctx.enter_context(tc.tile_pool(name="h0p", bufs=1))
                bt_ = bias.tile([1, G3], BF16, name="wih1b", tag="wih1b")
                nc.sync.dma_start(bt_[:, :], wih1T[H:H + 1, :])
                h0sb = h0p.tile([128, KT, T, B], BF16, name="h0sb",
                                tag="h0sb")
                for t in range(T):
                    nc.scalar.dma_start(h0sb[:, :, t, :], h0_tiles[t])
                for ch in range(NCH):
                    cs = slice(ch * CS, (ch + 1) * CS)
                    pts = [psum.tile([128, CS], F32, name="p1", tag="g1p")
                           for _ in range(MT)]
                    for ko in range(KT):
                        ks = slice(ko * 128, (ko + 1) * 128)
                        wt = stream.tile([128, G3], BF16, name="w1",
                                         tag="wst")
                        nc.sync.dma_start(wt[:, :CS], wih1T[ks, cs])
                        for m in range(MT):
                            nc.tensor.matmul(
                                pts[m][:, :], h0sb[:, ko, 2 * m:2 * m + 2, :],
                                wt[:, :CS], start=(ko == 0), stop=False)
                    for m in range(MT):
                        nc.tensor.matmul(     # bias row via K=1 ones matmul
                            pts[m][:, :], ones_sb[:, :128], bt_[:, cs],
                            start=False, stop=True)
                        ot = outp.tile([128, CS], BF16, name="o1", tag="g1o")
                        nc.vector.tensor_copy(ot[:, :], pts[m][:, :])
                        nc.sync.dma_start(
                            gi_d[1][m * 128:(m + 1) * 128, cs], ot[:, :])

        def recurrence(layer, wres):
            """T GRU steps for one layer."""
            with ExitStack() as ctx:
                psum = ctx.enter_context(
                    tc.tile_pool(name="ghpsum", bufs=NCH, space="PSUM"))
                trps = ctx.enter_context(
                    tc.tile_pool(name="trpsum", bufs=2, space="PSUM"))
                hsb_p = ctx.enter_context(tc.tile_pool(name="hsb", bufs=2))
                gi_p = ctx.enter_context(tc.tile_pool(name="gis", bufs=2))
                ew = ctx.enter_context(tc.tile_pool(name="ew", bufs=1))
                hn_p = ctx.enter_context(tc.tile_pool(name="hnew", bufs=2))
                tr_p = ctx.enter_context(tc.tile_pool(name="htr", bufs=2))

                bh = bhh_pool.tile([1, G3], BF16, name="bh", tag="bh")
                nc.sync.dma_start(bh[:, :], bhhn[layer][:, :])

                h_prev = None
                h_gathered = None   # DRAM AP of last AllGather output
                ag_tiles = []       # layer-0 AG outputs (for gi1)
                # chunk ch lives in PSUM tile ch//2, partition half ch%2 —
                # pairs of chunks share a bank and run in separate PE column
                # groups concurrently.
                nbank = (NCH + 1) // 2 if coltile else NCH

                def chunk_ap(pts, ch):
                    if coltile:
                        return pts[ch // 2][(ch % 2) * 64:(ch % 2) * 64 + 64,
                                            :]
                    return pts[ch][0:64, :]

                for t in range(T):
                    # ---- gh = h_{t-1} @ W_hh^T (+ bhh_n) in PSUM ----
                    pts = [psum.tile([128, CS] if coltile else [64, CS], F32,
                                     name="gh", tag="gh")
                           for _ in range(nbank)]
                    # Full-bank zero "seed" starts each bank's accumulation
                    # group: it spans both partition halves, so every later
                    # matmul overlaps it (Tile orders them after it) and
                    # accumulates onto zero via has_written set by the seed.
                    np_ = 128 if coltile else 64
                    for bk in range(nbank):
                        nc.tensor.matmul(pts[bk][:np_, :],
                                         ones_sb[:, :np_], zrow[:, :],
                                         start=True, stop=False)
                    if t > 0:
                        hsb = hsb_p.tile([128, KT, B], BF16, name="hsbt",
                                         tag="hsbt")
                        nc.scalar.dma_start(hsb[:, :, :], h_gathered)
                        for ko in range(KT):
                            if ko < R:
                                wt = wres[:, ko, :]
                            else:
                                wtile = stream.tile([128, G3], BF16,
                                                    name="ws", tag="wst")
                                nc.sync.dma_start(
                                    wtile[:, :],
                                    whhT[layer][ko * 128:(ko + 1) * 128, :])
                                wt = wtile[:, :]
                            for ch in range(NCH):
                                nc.tensor.matmul(
                                    chunk_ap(pts, ch), hsb[:, ko, :],
                                    wt[:, ch * CS:(ch + 1) * CS],
                                    start=False, stop=False)
                    for ch in range(NCH):
                        nc.tensor.matmul(
                            chunk_ap(pts, ch), ones_sb[:, :64],
                            bh[:, ch * CS:(ch + 1) * CS],
                            start=False, stop=False)
                    for bk in range(nbank):   # close each bank's group
                        nc.tensor.matmul(pts[bk][:np_, :],
                                         ones_sb[:, :np_], zrow[:, :],
                                         start=False, stop=True)

                    # ---- elementwise gates ----
                    gi_t = gi_p.tile([64, G3], BF16, name="git", tag="git")
                    nc.scalar.dma_start(
                        gi_t[:, :], gi_d[layer][t * 64:(t + 1) * 64, :])
                    s = ew.tile([64, 2 * PH], F32, name="s", tag="s")
                    rz = ew.tile([64, 2 * PH], F32, name="rz", tag="rz")
                    npre = ew.tile([64, PH], F32, name="npre", tag="npre")
                    nadd = ew.tile([64, PH], F32, name="nadd", tag="nadd")
                    nt = ew.tile([64, PH], F32, name="nt", tag="nt")
                    d = ew.tile([64, PH], F32, name="d", tag="d")
                    e = ew.tile([64, PH], F32, name="e", tag="e")
                    h_new = hn_p.tile([64, PH], F32, name="hnw", tag="hnw")
                    for ch in range(2 * NCHG):   # r and z chunks
                        cls = slice(ch * CS, (ch + 1) * CS)
                        nc.vector.tensor_add(s[:, cls], chunk_ap(pts, ch),
                                             gi_t[:, cls])
                    nc.scalar.activation(rz[:, :PH], s[:, :PH], ACT.Sigmoid)
                    nc.scalar.activation(rz[:, PH:], s[:, PH:], ACT.Sigmoid)
                    for j in range(NCHG):        # n chunks: r * gh_n
                        cls = slice(j * CS, (j + 1) * CS)
                        nc.vector.tensor_mul(npre[:, cls],
                                             chunk_ap(pts, 2 * NCHG + j),
                                             rz[:, cls])
                    nc.vector.tensor_add(nadd[:, :], npre[:, :],
                                         gi_t[:, 2 * PH:])
                    nc.scalar.activation(nt[:, :], nadd[:, :], ACT.Tanh)
                    if t == 0:
                        nc.vector.tensor_mul(e[:, :], rz[:, PH:], nt[:, :])
                        nc.vector.tensor_sub(h_new[:, :], nt[:, :], e[:, :])
                    else:
                        nc.vector.tensor_sub(d[:, :], h_prev[:, :], nt[:, :])
                        nc.vector.tensor_mul(e[:, :], rz[:, PH:], d[:, :])
                        nc.vector.tensor_add(h_new[:, :], nt[:, :], e[:, :])
                    h_prev = h_new

                    # ---- transpose h_new -> (PH,64): f32 out + bf16 AG ----
                    need_f32 = (layer == 1) or (t == T - 1)
                    hsh = tr_p.tile([128, len(cfg.tr), B], BF16, name="hsh",
                                    tag="hsh")
                    if need_f32:
                        htr = tr_p.tile([128, len(cfg.tr), B], F32,
                                        name="htf", tag="htf")
                    for j, (off, sz) in enumerate(cfg.tr):
                        tp = trps.tile([128, 64], F32, name="tp", tag="tp")
                        nc.tensor.transpose(tp[:sz, :],
                                            h_new[:, off:off + sz],
                                            ident[:, :])
                        if need_f32:
                            nc.vector.tensor_copy(htr[:sz, j, :], tp[:sz, :])
                        nc.vector.tensor_copy(hsh[:sz, j, :], tp[:sz, :])

                    if layer == 1:
                        for j, (off, sz) in enumerate(cfg.tr):
                            nc.sync.dma_start(out1[t, off:off + sz, :],
                                              htr[:sz, j, :])
                    if layer == 0 and t == T - 1:
                        for j, (off, sz) in enumerate(cfg.tr):
                            nc.sync.dma_start(hid0[off:off + sz, :],
                                              htr[:sz, j, :])

                    # ---- AllGather h across cores ----
                    if layer == 1 and t == T - 1:
                        continue
                    # local shard (PH, B) in pl-major local-unit order; the
                    # 8 shards concatenate to the packed (128, KT, B) layout.
                    hshard = dram2.tile([PH, B], BF16, name="hshd",
                                        tag="hshd")
                    for j, (off, sz) in enumerate(cfg.tr):
                        nc.scalar.dma_start(hshard[off:off + sz, :],
                                            hsh[:sz, j, :])
                    hc = dram2.tile([128, KT, B], BF16, name="hc", tag="hc",
                                    bufs=max(T, 2), addr_space="Shared")
                    nc.gpsimd.collective_compute(
                        "AllGather", mybir.AluOpType.bypass,
                        replica_groups=rg,
                        ins=[hshard[:, :].opt()],
                        outs=[hc[:, :, :].opt()],
                    )
                    if layer == 0:
                        ag_tiles.append(hc[:, :, :])
                    h_gathered = hc[:, :, :]
                return ag_tiles

        # ---- phase structure ----
        gi0_gemm()
        wres0 = load_resident(0)
        h0_tiles = recurrence(0, wres0)
        gi1_gemm(h0_tiles)
        wres1 = load_resident(1)
        recurrence(1, wres1)

    nc.compile()
    return nc


# --------------------------------------------------------------------------
# host side
# --------------------------------------------------------------------------

def _unit_order(cfg: Cfg, c: int) -> np.ndarray:
    """Global hidden-unit index for core c's local units 0..PH-1.
    Local order is pl-major: j = pl*KT + ko -> unit ko*128 + c*PL + pl."""
    ar = np.arange(cfg.PH)
    return (ar % cfg.KT) * 128 + c * PL + (ar // cfg.KT)


def prep_inputs(cfg: Cfg, inputs, W_ih0, W_hh0, b_ih0, b_hh0,
                W_ih1, W_hh1, b_ih1, b_hh1):
    B, T, H, PH, IN0 = cfg.B, cfg.T, cfg.H, cfg.PH, cfg.IN0
    inputs = np.asarray(inputs, np.float32)
    assert inputs.shape == (B, T, IN0)
    xT = np.empty((IN0 + 1, cfg.BT), dtype=np.float32)
    xT[:IN0] = inputs.transpose(2, 1, 0).reshape(IN0, T * B)  # col = t*B+b
    xT[IN0] = 1.0
    xT = xT.astype(BF16NP)

    arrs = {k: np.asarray(v, np.float32) for k, v in dict(
        W_ih0=W_ih0, W_hh0=W_hh0, b_ih0=b_ih0, b_hh0=b_hh0,
        W_ih1=W_ih1, W_hh1=W_hh1, b_ih1=b_ih1, b_hh1=b_hh1).items()}

    def shard(c):
        unit = _unit_order(cfg, c)
        idx = np.concatenate([g * H + unit for g in range(3)])

        def wt(W, b_i, b_h, kdim):
            out = np.empty((kdim + 1, 3 * PH), dtype=np.float32)
            out[:kdim] = W[idx].T
            bias = b_i[idx].copy()
            bias[:2 * PH] += b_h[idx][:2 * PH]   # r,z: b_ih+b_hh; n: b_ih
            out[kdim] = bias
            return out.astype(BF16NP)

        m = {
            "xT": xT,
            "wih0T": wt(arrs["W_ih0"], arrs["b_ih0"], arrs["b_hh0"], IN0),
            "wih1T": wt(arrs["W_ih1"], arrs["b_ih1"], arrs["b_hh1"], H),
            "whh0T": np.ascontiguousarray(arrs["W_hh0"][idx].T).astype(BF16NP),
            "whh1T": np.ascontiguousarray(arrs["W_hh1"][idx].T).astype(BF16NP),
        }
        for l in range(2):
            row = np.zeros((1, 3 * PH), dtype=np.float32)
            row[0, 2 * PH:] = arrs[f"b_hh{l}"][idx][2 * PH:]
            m[f"bhhn{l}"] = row.astype(BF16NP)
        return m

    return [shard(c) for c in range(NCORES)]


def assemble(cfg: Cfg, outs):
    T, B, H, KT = cfg.T, cfg.B, cfg.H, cfg.KT
    # local unit j=(pl*KT+ko) on core c -> global unit ko*128 + c*PL + pl
    out1 = np.stack([outs[c]["out1"] for c in range(NCORES)])  # (8,T,PH,B)
    out1 = out1.reshape(NCORES, T, PL, KT, B)
    output = np.ascontiguousarray(
        out1.transpose(1, 4, 3, 0, 2).reshape(T, B, H))
    hid0 = np.stack([outs[c]["hid0"] for c in range(NCORES)])  # (8,PH,B)
    hid0 = hid0.reshape(NCORES, PL, KT, B)
    h0_last = hid0.transpose(3, 2, 0, 1).reshape(B, H)
    hidden = np.stack([h0_last, output[T - 1]], axis=0)
    return output.astype(np.float32), hidden.astype(np.float32)


LAST_RESULTS = None


def _ensure_ntff_hook():
    """The agent image's antenv lacks axon_hooks; recreate it so
    trace=True can drive NTFF profiling via the injected libaxon so."""
    try:
        from antenv.axon_hooks import get_axon_ntff_profile_hook  # noqa: F401
        return
    except ImportError:
        pass
    import sys
    import types

    import antenv

    mod = types.ModuleType("antenv.axon_hooks")
    _hook = [None]
    mod.set_axon_ntff_profile_hook = lambda h: _hook.__setitem__(0, h)
    mod.get_axon_ntff_profile_hook = lambda: _hook[0]
    sys.modules["antenv.axon_hooks"] = mod
    antenv.axon_hooks = mod
    try:
        if "/root/.axon_site" not in sys.path:
            sys.path.insert(0, "/root/.axon_site")
        from trn_agent_boot.trn_boot import _ntff_profile_via_ctypes
        so = "/opt/axon/libaxon_pjrt.so"
        if os.path.exists(so):
            mod.set_axon_ntff_profile_hook(_ntff_profile_via_ctypes(so))
    except Exception:
        pass


def kernel(inputs, W_ih0, W_hh0, b_ih0, b_hh0, W_ih1, W_hh1, b_ih1, b_hh1):
    global LAST_RESULTS
    cfg = Cfg(R=int(os.environ.get("GRU_R", "14")),
              stream_bufs=int(os.environ.get("GRU_SB", "5")))
    nc = build_nc(cfg)
    in_maps = prep_inputs(cfg, inputs, W_ih0, W_hh0, b_ih0, b_hh0,
                          W_ih1, W_hh1, b_ih1, b_hh1)
    trace = bool(int(os.environ.get("GRU_TRACE", "0")))
    if trace:
        _ensure_ntff_hook()
    res = run_bass_kernel_spmd(nc, in_maps, list(range(NCORES)), trace=trace)
    LAST_RESULTS = res
    return assemble(cfg, res.results)


# revision 31
# speedup vs baseline: 1.3371x; 1.0325x over previous
"""Trainium2 Bass kernel for nn_EncoderModel (2-layer GRU encoder).

Model: B=64, T=12, H=6400 (3H=19200 gate rows), IN0=200, 2 layers.

Sharding: tensor-parallel over the gate/output dim with an interleaved
unit map: core c owns hidden units {ko*128 + c*16 + pl} (16 partition rows
of every 128-row K-tile), for each of the r/z/n gate blocks. Each step every
core computes its 800-unit slice of the new hidden state; the AllGather of
the 8 per-core (16, KT, B) shards then reconstructs the full h directly in
the packed (128 partitions, KT, B) SBUF lhsT layout — one contiguous DMA.

Per-core compute layouts:
  - hh GEMM: out = h @ W^T accumulated in PSUM as (batch=64 part, gate free);
    lhsT = h^T K-tiles (128,64) stationary, rhs = W^T K-tiles (128,CS) moving.
    PE column-tiling 2x: even K-tiles accumulate into PSUM partitions 0:64,
    odd K-tiles into 64:128 (concurrent in separate PE column groups); the
    two halves are summed by the DVE at gate-evaluation time.
  - W^T (H, 3*PH) bf16 is streamed from HBM each step; first R K-tiles stay
    resident in SBUF.
  - h_new (64,PH) f32 -> PE-transposed to (PH,64), cast bf16 -> AllGather.
  - All biases are folded into the gi GEMM (ones row in lhsT, bias row in
    W_ih^T), except b_hh[n-gate], which joins gh_n via a K=1 ones-row matmul
    so n = tanh(gi_n + r*(h@W_hh_n^T + b_hh_n)) matches the reference.
"""

import os
from contextlib import ExitStack

import ml_dtypes
import numpy as np

import concourse.bass as bass
import concourse.mybir as mybir
import concourse.tile as tile
from concourse import bacc
from concourse.bass_utils import run_bass_kernel_spmd
from concourse.masks import make_identity

F32 = mybir.dt.float32
BF16 = mybir.dt.bfloat16
BF16NP = ml_dtypes.bfloat16
ACT = mybir.ActivationFunctionType

NCORES = 8
PL = 128 // NCORES   # partition rows per core within each K-tile


class Cfg:
    def __init__(self, B=64, T=12, H=6400, IN0=200, R=16, stream_bufs=5):
        self.B, self.T, self.H, self.IN0 = B, T, H, IN0
        assert B == 64
        assert H % 128 == 0
        self.PH = H // NCORES            # hidden units per core
        self.G3 = 3 * self.PH            # gate rows per core
        self.KT = H // 128               # K tiles over H
        assert self.PH == self.KT * PL
        self.CS = 400 if self.PH % 400 == 0 else self.PH   # psum chunk size
        assert self.PH % self.CS == 0 and self.CS <= 512
        self.NCHG = self.PH // self.CS   # chunks per gate
        self.NCH = 3 * self.NCHG         # psum chunks per step
        assert self.NCH + 2 <= 8, "psum banks"
        self.BT = B * T
        assert self.BT % 128 == 0
        self.MT = self.BT // 128         # m tiles for gi GEMMs
        self.R = min(R, self.KT)         # resident W K-tiles
        self.stream_bufs = stream_bufs
        # K tiling for IN0+1 (ones/bias row folded in)
        k, off, self.in0_ks = IN0 + 1, 0, []
        while off < k:
            s = min(128, k - off)
            self.in0_ks.append((off, s))
            off += s
        # transpose tiles over PH (each tile covers sz//PL K-tiles)
        off, self.tr = 0, []
        while off < self.PH:
            s = min(128, self.PH - off)
            assert s % PL == 0
            self.tr.append((off, s))
            off += s


def build_nc(cfg: Cfg) -> bass.Bass:
    B, T, H, PH, G3 = cfg.B, cfg.T, cfg.H, cfg.PH, cfg.G3
    KT, CS, NCH, NCHG, MT, R = cfg.KT, cfg.CS, cfg.NCH, cfg.NCHG, cfg.MT, cfg.R
    BT = cfg.BT
    coltile = KT >= 2
    last_odd = KT - 1 if (KT - 1) % 2 == 1 else KT - 2

    nc = bacc.Bacc("TRN2", target_bir_lowering=False, debug=False,
                   num_devices=NCORES)
    rg = [list(range(NCORES))]

    # ---- kernel I/O (per-core data via in_maps) ----
    xT = nc.dram_tensor("xT", [cfg.IN0 + 1, BT], BF16, kind="ExternalInput")
    wih0T = nc.dram_tensor("wih0T", [cfg.IN0 + 1, G3], BF16,
                           kind="ExternalInput")
    wih1T = nc.dram_tensor("wih1T", [H + 1, G3], BF16, kind="ExternalInput")
    whh0T = nc.dram_tensor("whh0T", [H, G3], BF16, kind="ExternalInput")
    whh1T = nc.dram_tensor("whh1T", [H, G3], BF16, kind="ExternalInput")
    bhhn0 = nc.dram_tensor("bhhn0", [1, G3], BF16, kind="ExternalInput")
    bhhn1 = nc.dram_tensor("bhhn1", [1, G3], BF16, kind="ExternalInput")
    out1 = nc.dram_tensor("out1", [T, PH, B], F32, kind="ExternalOutput")
    hid0 = nc.dram_tensor("hid0", [PH, B], F32, kind="ExternalOutput")
    whhT = [whh0T, whh1T]
    bhhn = [bhhn0, bhhn1]

    with tile.TileContext(nc) as tc, ExitStack() as top:
        # ---- persistent pools ----
        dram = top.enter_context(tc.tile_pool(name="dram", bufs=1,
                                              space="DRAM"))
        dram2 = top.enter_context(tc.tile_pool(name="dram2", bufs=2,
                                               space="DRAM"))
        consts = top.enter_context(tc.tile_pool(name="consts", bufs=1))
        res_pool = top.enter_context(tc.tile_pool(name="wres", bufs=1))
        stream = top.enter_context(
            tc.tile_pool(name="wstream", bufs=cfg.stream_bufs))
        ktile_pool = top.enter_context(tc.tile_pool(name="ktiles", bufs=3))
        bhh_pool = top.enter_context(tc.tile_pool(name="bhhp", bufs=1))

        # DRAM intermediates
        gi_d = [dram.tile([BT, G3], BF16, name=f"gi{l}", tag=f"gi{l}")
                for l in range(2)]

        # constants
        ident = consts.tile([64, 64], F32, name="ident", tag="ident")
        make_identity(nc, ident[:, :])
        ones_sb = consts.tile([1, 128], BF16, name="ones_sb", tag="ones_sb")
        nc.gpsimd.memset(ones_sb[:, :], 1.0)
        zrow = consts.tile([1, CS], BF16, name="zrow", tag="zrow")
        nc.gpsimd.memset(zrow[:, :], 0.0)

        def hwq(i):
            """Alternate bulk streams across the two HW-DGE queues."""
            return nc.sync if i % 2 == 0 else nc.scalar

        def load_resident(layer):
            wres = res_pool.tile([128, max(R, 1), G3], BF16, name="wres",
                                 tag="wres")
            for ko in range(R):
                hwq(ko).dma_start(wres[:, ko, :],
                                  whhT[layer][ko * 128:(ko + 1) * 128, :])
            return wres

        def gi0_gemm():
            """gi0 = [x;1]^T @ [W_ih0^T;bias] -> gi_d[0] (BT,G3) bf16."""
            with ExitStack() as ctx:
                psum = ctx.enter_context(
                    tc.tile_pool(name="gi0psum", bufs=4, space="PSUM"))
                outp = ctx.enter_context(tc.tile_pool(name="gi0out", bufs=4))
                xts, wts = [], []
                for i, (off, sz) in enumerate(cfg.in0_ks):
                    xt = ktile_pool.tile([128, BT], BF16, name=f"x{i}",
                                         tag="kx")
                    nc.sync.dma_start(xt[:sz, :], xT[off:off + sz, :])
                    xts.append(xt)
                    wt = stream.tile([128, G3], BF16, name=f"w{i}", tag="wst")
                    nc.sync.dma_start(wt[:sz, :], wih0T[off:off + sz, :])
                    wts.append(wt)
                for ch in range(NCH):
                    cs = slice(ch * CS, (ch + 1) * CS)
                    for m in range(MT):
                        ms = slice(m * 128, (m + 1) * 128)
                        pt = psum.tile([128, CS], F32, name="pt", tag="gp")
                        for i, (off, sz) in enumerate(cfg.in0_ks):
                            nc.tensor.matmul(
                                pt[:, :], xts[i][:sz, ms], wts[i][:sz, cs],
                                start=(i == 0),
                                stop=(i == len(cfg.in0_ks) - 1))
                        ot = outp.tile([128, CS], BF16, name="ot", tag="go")
                        nc.vector.tensor_copy(ot[:, :], pt[:, :])
                        nc.sync.dma_start(gi_d[0][ms, cs], ot[:, :])

        def gi1_gemm(h0_tiles):
            """gi1 = [h0;1]^T @ [W_ih1^T;bias] -> gi_d[1] (BT,G3) bf16.

            h0_tiles: layer-0's T AllGather outputs, each (128, KT, B).
            All of h0 is preloaded into SBUF once (T contiguous DMAs)."""
            with ExitStack() as ctx:
                psum = ctx.enter_context(
                    tc.tile_pool(name="gi1psum", bufs=MT, space="PSUM"))
                outp = ctx.enter_context(tc.tile_pool(name="gi1out", bufs=4))
                bias = ctx.enter_context(tc.tile_pool(name="gi1bias", bufs=1))
                bt_ = bias.tile([1, G3], BF16, name="wih1b", tag="wih1b")
                nc.sync.dma_start(bt_[:, :], wih1T[H:H + 1, :])
                # h0 staging shares the weight-residency slot (layer-0's
                # resident weights are dead here; layer-1's load comes after)
                h0sb = res_pool.tile([128, KT, T, B], BF16, name="h0sb",
                                     tag="wres")
                for t in range(T):
                    nc.gpsimd.dma_start(h0sb[:, :, t, :], h0_tiles[t])
                for ch in range(NCH):
                    cs = slice(ch * CS, (ch + 1) * CS)
                    pts = [psum.tile([128, CS], F32, name="p1", tag="g1p")
                           for _ in range(MT)]
                    for ko in range(KT):
                        ks = slice(ko * 128, (ko + 1) * 128)
                        wt = stream.tile([128, G3], BF16, name="w1",
                                         tag="wst")
                        hwq(ko).dma_start(wt[:, :CS], wih1T[ks, cs])
                        for m in range(MT):
                            nc.tensor.matmul(
                                pts[m][:, :], h0sb[:, ko, 2 * m:2 * m + 2, :],
                                wt[:, :CS], start=(ko == 0), stop=False)
                    for m in range(MT):
                        nc.tensor.matmul(     # bias row via K=1 ones matmul
                            pts[m][:, :], ones_sb[:, :128], bt_[:, cs],
                            start=False, stop=True)
                        ot = outp.tile([128, CS], BF16, name="o1", tag="g1o")
                        nc.vector.tensor_copy(ot[:, :], pts[m][:, :])
                        nc.sync.dma_start(
                            gi_d[1][m * 128:(m + 1) * 128, cs], ot[:, :])

        def recurrence(layer, wres):
            """T GRU steps for one layer."""
            with ExitStack() as ctx:
                psum = ctx.enter_context(
                    tc.tile_pool(name="ghpsum", bufs=NCH, space="PSUM"))
                trps = ctx.enter_context(
                    tc.tile_pool(name="trpsum", bufs=2, space="PSUM"))
                hsb_p = ctx.enter_context(tc.tile_pool(name="hsb", bufs=2))
                gi_p = ctx.enter_context(tc.tile_pool(name="gis", bufs=2))
                ew = ctx.enter_context(tc.tile_pool(name="ew", bufs=1))
                hn_p = ctx.enter_context(tc.tile_pool(name="hnew", bufs=2))
                tr_p = ctx.enter_context(tc.tile_pool(name="htr", bufs=2))

                bh = bhh_pool.tile([1, G3], BF16, name="bh", tag="bh")
                nc.sync.dma_start(bh[:, :], bhhn[layer][:, :])

                h_prev = None
                h_gathered = None   # DRAM AP of last AllGather output
                ag_tiles = []       # layer-0 AG outputs (for gi1)
                # chunk ch lives in PSUM tile ch//2, partition half ch%2 —
                # pairs of chunks share a bank and run in separate PE column
                # groups concurrently.
                nbank = (NCH + 1) // 2 if coltile else NCH

                def chunk_ap(pts, ch):
                    if coltile:
                        return pts[ch // 2][(ch % 2) * 64:(ch % 2) * 64 + 64,
                                            :]
                    return pts[ch][0:64, :]

                for t in range(T):
                    # ---- gh = h_{t-1} @ W_hh^T (+ bhh_n) in PSUM ----
                    pts = [psum.tile([128, CS] if coltile else [64, CS], F32,
                                     name="gh", tag="gh")
                           for _ in range(nbank)]
                    # Full-bank zero "seed" starts each bank's accumulation
                    # group: it spans both partition halves, so every later
                    # matmul overlaps it (Tile orders them after it) and
                    # accumulates onto zero via has_written set by the seed.
                    np_ = 128 if coltile else 64
                    for bk in range(nbank):
                        nc.tensor.matmul(pts[bk][:np_, :],
                                         ones_sb[:, :np_], zrow[:, :],
                                         start=True, stop=False)
                    if t > 0:
                        hsb = hsb_p.tile([128, KT, B], BF16, name="hsbt",
                                         tag="hsbt")
                        nc.gpsimd.dma_start(hsb[:, :, :], h_gathered)
                        for ko in range(KT):
                            if ko < R:
                                wt = wres[:, ko, :]
                            else:
                                wtile = stream.tile([128, G3], BF16,
                                                    name="ws", tag="wst")
                                hwq(ko).dma_start(
                                    wtile[:, :],
                                    whhT[layer][ko * 128:(ko + 1) * 128, :])
                                wt = wtile[:, :]
                            for ch in range(NCH):
                                nc.tensor.matmul(
                                    chunk_ap(pts, ch), hsb[:, ko, :],
                                    wt[:, ch * CS:(ch + 1) * CS],
                                    start=False, stop=False)
                    for ch in range(NCH):
                        nc.tensor.matmul(
                            chunk_ap(pts, ch), ones_sb[:, :64],
                            bh[:, ch * CS:(ch + 1) * CS],
                            start=False, stop=False)
                    for bk in range(nbank):   # close each bank's group
                        nc.tensor.matmul(pts[bk][:np_, :],
                                         ones_sb[:, :np_], zrow[:, :],
                                         start=False, stop=True)

                    # ---- elementwise gates ----
                    gi_t = gi_p.tile([64, G3], BF16, name="git", tag="git")
                    nc.gpsimd.dma_start(
                        gi_t[:, :], gi_d[layer][t * 64:(t + 1) * 64, :])
                    s = ew.tile([64, 2 * PH], F32, name="s", tag="s")
                    rz = ew.tile([64, 2 * PH], F32, name="rz", tag="rz")
                    npre = ew.tile([64, PH], F32, name="npre", tag="npre")
                    nadd = ew.tile([64, PH], F32, name="nadd", tag="nadd")
                    nt = ew.tile([64, PH], F32, name="nt", tag="nt")
                    d = ew.tile([64, PH], F32, name="d", tag="d")
                    e = ew.tile([64, PH], F32, name="e", tag="e")
                    h_new = hn_p.tile([64, PH], F32, name="hnw", tag="hnw")
                    for ch in range(2 * NCHG):   # r and z chunks
                        cls = slice(ch * CS, (ch + 1) * CS)
                        nc.vector.tensor_add(s[:, cls], chunk_ap(pts, ch),
                                             gi_t[:, cls])
                    nc.scalar.activation(rz[:, :PH], s[:, :PH], ACT.Sigmoid)
                    nc.scalar.activation(rz[:, PH:], s[:, PH:], ACT.Sigmoid)
                    for j in range(NCHG):        # n chunks: r * gh_n
                        cls = slice(j * CS, (j + 1) * CS)
                        nc.vector.tensor_mul(npre[:, cls],
                                             chunk_ap(pts, 2 * NCHG + j),
                                             rz[:, cls])
                    nc.vector.tensor_add(nadd[:, :], npre[:, :],
                                         gi_t[:, 2 * PH:])
                    nc.scalar.activation(nt[:, :], nadd[:, :], ACT.Tanh)
                    if t == 0:
                        nc.vector.tensor_mul(e[:, :], rz[:, PH:], nt[:, :])
                        nc.vector.tensor_sub(h_new[:, :], nt[:, :], e[:, :])
                    else:
                        nc.vector.tensor_sub(d[:, :], h_prev[:, :], nt[:, :])
                        nc.vector.tensor_mul(e[:, :], rz[:, PH:], d[:, :])
                        nc.vector.tensor_add(h_new[:, :], nt[:, :], e[:, :])
                    h_prev = h_new

                    # ---- transpose h_new -> (PH,64): f32 out + bf16 AG ----
                    need_f32 = (layer == 1) or (t == T - 1)
                    hsh = tr_p.tile([128, len(cfg.tr), B], BF16, name="hsh",
                                    tag="hsh")
                    if need_f32:
                        htr = tr_p.tile([128, len(cfg.tr), B], F32,
                                        name="htf", tag="htf")
                    for j, (off, sz) in enumerate(cfg.tr):
                        tp = trps.tile([128, 64], F32, name="tp", tag="tp")
                        nc.tensor.transpose(tp[:sz, :],
                                            h_new[:, off:off + sz],
                                            ident[:, :])
                        if need_f32:
                            nc.vector.tensor_copy(htr[:sz, j, :], tp[:sz, :])
                        nc.vector.tensor_copy(hsh[:sz, j, :], tp[:sz, :])

                    if layer == 1:
                        for j, (off, sz) in enumerate(cfg.tr):
                            nc.gpsimd.dma_start(out1[t, off:off + sz, :],
                                              htr[:sz, j, :])
                    if layer == 0 and t == T - 1:
                        for j, (off, sz) in enumerate(cfg.tr):
                            nc.gpsimd.dma_start(hid0[off:off + sz, :],
                                              htr[:sz, j, :])

                    # ---- AllGather h across cores ----
                    if layer == 1 and t == T - 1:
                        continue
                    # local shard (PH, B) in pl-major local-unit order; the
                    # 8 shards concatenate to the packed (128, KT, B) layout.
                    hshard = dram2.tile([PH, B], BF16, name="hshd",
                                        tag="hshd")
                    for j, (off, sz) in enumerate(cfg.tr):
                        nc.gpsimd.dma_start(hshard[off:off + sz, :],
                                            hsh[:sz, j, :])
                    hc = dram2.tile([128, KT, B], BF16, name="hc", tag="hc",
                                    bufs=max(T, 2), addr_space="Shared")
                    nc.gpsimd.collective_compute(
                        "AllGather", mybir.AluOpType.bypass,
                        replica_groups=rg,
                        ins=[hshard[:, :].opt()],
                        outs=[hc[:, :, :].opt()],
                    )
                    if layer == 0:
                        ag_tiles.append(hc[:, :, :])
                    h_gathered = hc[:, :, :]
                return ag_tiles

        # ---- phase structure ----
        gi0_gemm()
        wres0 = load_resident(0)
        h0_tiles = recurrence(0, wres0)
        gi1_gemm(h0_tiles)
        wres1 = load_resident(1)
        recurrence(1, wres1)

    nc.compile()
    return nc


# --------------------------------------------------------------------------
# host side
# --------------------------------------------------------------------------

def _unit_order(cfg: Cfg, c: int) -> np.ndarray:
    """Global hidden-unit index for core c's local units 0..PH-1.
    Local order is pl-major: j = pl*KT + ko -> unit ko*128 + c*PL + pl."""
    ar = np.arange(cfg.PH)
    return (ar % cfg.KT) * 128 + c * PL + (ar // cfg.KT)


def prep_inputs(cfg: Cfg, inputs, W_ih0, W_hh0, b_ih0, b_hh0,
                W_ih1, W_hh1, b_ih1, b_hh1):
    B, T, H, PH, IN0 = cfg.B, cfg.T, cfg.H, cfg.PH, cfg.IN0
    inputs = np.asarray(inputs, np.float32)
    assert inputs.shape == (B, T, IN0)
    xT = np.empty((IN0 + 1, cfg.BT), dtype=np.float32)
    xT[:IN0] = inputs.transpose(2, 1, 0).reshape(IN0, T * B)  # col = t*B+b
    xT[IN0] = 1.0
    xT = xT.astype(BF16NP)

    arrs = {k: np.asarray(v, np.float32) for k, v in dict(
        W_ih0=W_ih0, W_hh0=W_hh0, b_ih0=b_ih0, b_hh0=b_hh0,
        W_ih1=W_ih1, W_hh1=W_hh1, b_ih1=b_ih1, b_hh1=b_hh1).items()}

    def shard(c):
        unit = _unit_order(cfg, c)
        idx = np.concatenate([g * H + unit for g in range(3)])

        def wt(W, b_i, b_h, kdim):
            out = np.empty((kdim + 1, 3 * PH), dtype=np.float32)
            out[:kdim] = W[idx].T
            bias = b_i[idx].copy()
            bias[:2 * PH] += b_h[idx][:2 * PH]   # r,z: b_ih+b_hh; n: b_ih
            out[kdim] = bias
            return out.astype(BF16NP)

        m = {
            "xT": xT,
            "wih0T": wt(arrs["W_ih0"], arrs["b_ih0"], arrs["b_hh0"], IN0),
            "wih1T": wt(arrs["W_ih1"], arrs["b_ih1"], arrs["b_hh1"], H),
            "whh0T": np.ascontiguousarray(arrs["W_hh0"][idx].T).astype(BF16NP),
            "whh1T": np.ascontiguousarray(arrs["W_hh1"][idx].T).astype(BF16NP),
        }
        for l in range(2):
            row = np.zeros((1, 3 * PH), dtype=np.float32)
            row[0, 2 * PH:] = arrs[f"b_hh{l}"][idx][2 * PH:]
            m[f"bhhn{l}"] = row.astype(BF16NP)
        return m

    return [shard(c) for c in range(NCORES)]


def assemble(cfg: Cfg, outs):
    T, B, H, KT = cfg.T, cfg.B, cfg.H, cfg.KT
    # local unit j=(pl*KT+ko) on core c -> global unit ko*128 + c*PL + pl
    out1 = np.stack([outs[c]["out1"] for c in range(NCORES)])  # (8,T,PH,B)
    out1 = out1.reshape(NCORES, T, PL, KT, B)
    output = np.ascontiguousarray(
        out1.transpose(1, 4, 3, 0, 2).reshape(T, B, H))
    hid0 = np.stack([outs[c]["hid0"] for c in range(NCORES)])  # (8,PH,B)
    hid0 = hid0.reshape(NCORES, PL, KT, B)
    h0_last = hid0.transpose(3, 2, 0, 1).reshape(B, H)
    hidden = np.stack([h0_last, output[T - 1]], axis=0)
    return output.astype(np.float32), hidden.astype(np.float32)


LAST_RESULTS = None


def _ensure_ntff_hook():
    """The agent image's antenv lacks axon_hooks; recreate it so
    trace=True can drive NTFF profiling via the injected libaxon so."""
    try:
        from antenv.axon_hooks import get_axon_ntff_profile_hook  # noqa: F401
        return
    except ImportError:
        pass
    import sys
    import types

    import antenv

    mod = types.ModuleType("antenv.axon_hooks")
    _hook = [None]
    mod.set_axon_ntff_profile_hook = lambda h: _hook.__setitem__(0, h)
    mod.get_axon_ntff_profile_hook = lambda: _hook[0]
    sys.modules["antenv.axon_hooks"] = mod
    antenv.axon_hooks = mod
    try:
        if "/root/.axon_site" not in sys.path:
            sys.path.insert(0, "/root/.axon_site")
        from trn_agent_boot.trn_boot import _ntff_profile_via_ctypes
        so = "/opt/axon/libaxon_pjrt.so"
        if os.path.exists(so):
            mod.set_axon_ntff_profile_hook(_ntff_profile_via_ctypes(so))
    except Exception:
        pass


def kernel(inputs, W_ih0, W_hh0, b_ih0, b_hh0, W_ih1, W_hh1, b_ih1, b_hh1):
    global LAST_RESULTS
    cfg = Cfg(R=int(os.environ.get("GRU_R", "22")),
              stream_bufs=int(os.environ.get("GRU_SB", "5")))
    nc = build_nc(cfg)
    in_maps = prep_inputs(cfg, inputs, W_ih0, W_hh0, b_ih0, b_hh0,
                          W_ih1, W_hh1, b_ih1, b_hh1)
    trace = bool(int(os.environ.get("GRU_TRACE", "0")))
    if trace:
        _ensure_ntff_hook()
    res = run_bass_kernel_spmd(nc, in_maps, list(range(NCORES)), trace=trace)
    LAST_RESULTS = res
    return assemble(cfg, res.results)


# revision 35
# speedup vs baseline: 1.4362x; 1.0742x over previous
"""Trainium2 Bass kernel for nn_EncoderModel (2-layer GRU encoder).

Model: B=64, T=12, H=6400 (3H=19200 gate rows), IN0=200, 2 layers.

Sharding: tensor-parallel over the gate/output dim with an interleaved
unit map: core c owns hidden units {ko*128 + c*16 + pl} (16 partition rows
of every 128-row K-tile), for each of the r/z/n gate blocks. Each step every
core computes its 800-unit slice of the new hidden state; the AllGather of
the 8 per-core (16, KT, B) shards then reconstructs the full h directly in
the packed (128 partitions, KT, B) SBUF lhsT layout — one contiguous DMA.

Per-core compute layouts:
  - hh GEMM: out = h @ W^T accumulated in PSUM as (batch=64 part, gate free);
    lhsT = h^T K-tiles (128,64) stationary, rhs = W^T K-tiles (128,CS) moving.
    PE column-tiling 2x: even K-tiles accumulate into PSUM partitions 0:64,
    odd K-tiles into 64:128 (concurrent in separate PE column groups); the
    two halves are summed by the DVE at gate-evaluation time.
  - W^T (H, 3*PH) bf16 is streamed from HBM each step; first R K-tiles stay
    resident in SBUF.
  - h_new (64,PH) f32 -> PE-transposed to (PH,64), cast bf16 -> AllGather.
  - All biases are folded into the gi GEMM (ones row in lhsT, bias row in
    W_ih^T), except b_hh[n-gate], which joins gh_n via a K=1 ones-row matmul
    so n = tanh(gi_n + r*(h@W_hh_n^T + b_hh_n)) matches the reference.
"""

import os
from contextlib import ExitStack

import ml_dtypes
import numpy as np

import concourse.bass as bass
import concourse.mybir as mybir
import concourse.tile as tile
from concourse import bacc
from concourse.bass_utils import run_bass_kernel_spmd
from concourse.masks import make_identity

F32 = mybir.dt.float32
BF16 = mybir.dt.bfloat16
BF16NP = ml_dtypes.bfloat16
ACT = mybir.ActivationFunctionType

NCORES = 8
PL = 128 // NCORES   # partition rows per core within each K-tile


class Cfg:
    def __init__(self, B=64, T=12, H=6400, IN0=200, R=16, stream_bufs=5):
        self.B, self.T, self.H, self.IN0 = B, T, H, IN0
        assert B == 64
        assert H % 128 == 0
        self.PH = H // NCORES            # hidden units per core
        self.G3 = 3 * self.PH            # gate rows per core
        self.KT = H // 128               # K tiles over H
        assert self.PH == self.KT * PL
        self.CS = 400 if self.PH % 400 == 0 else self.PH   # psum chunk size
        assert self.PH % self.CS == 0 and self.CS <= 512
        self.NCHG = self.PH // self.CS   # chunks per gate
        self.NCH = 3 * self.NCHG         # psum chunks per step
        assert self.NCH + 2 <= 8, "psum banks"
        self.BT = B * T
        assert self.BT % 128 == 0
        self.MT = self.BT // 128         # m tiles for gi GEMMs
        self.R = min(R, self.KT)         # resident W K-tiles
        self.stream_bufs = stream_bufs
        # K tiling for IN0+1 (ones/bias row folded in)
        k, off, self.in0_ks = IN0 + 1, 0, []
        while off < k:
            s = min(128, k - off)
            self.in0_ks.append((off, s))
            off += s
        # transpose tiles over PH (each tile covers sz//PL K-tiles)
        off, self.tr = 0, []
        while off < self.PH:
            s = min(128, self.PH - off)
            assert s % PL == 0
            self.tr.append((off, s))
            off += s


def build_nc(cfg: Cfg) -> bass.Bass:
    B, T, H, PH, G3 = cfg.B, cfg.T, cfg.H, cfg.PH, cfg.G3
    KT, CS, NCH, NCHG, MT, R = cfg.KT, cfg.CS, cfg.NCH, cfg.NCHG, cfg.MT, cfg.R
    BT = cfg.BT
    coltile = KT >= 2
    last_odd = KT - 1 if (KT - 1) % 2 == 1 else KT - 2

    nc = bacc.Bacc("TRN2", target_bir_lowering=False, debug=False,
                   num_devices=NCORES)
    rg = [list(range(NCORES))]

    # ---- kernel I/O (per-core data via in_maps) ----
    xT = nc.dram_tensor("xT", [cfg.IN0 + 1, BT], BF16, kind="ExternalInput")
    wih0T = nc.dram_tensor("wih0T", [cfg.IN0 + 1, G3], BF16,
                           kind="ExternalInput")
    wih1T = nc.dram_tensor("wih1T", [H + 1, G3], BF16, kind="ExternalInput")
    whh0T = nc.dram_tensor("whh0T", [H, G3], BF16, kind="ExternalInput")
    whh1T = nc.dram_tensor("whh1T", [H, G3], BF16, kind="ExternalInput")
    bhhn0 = nc.dram_tensor("bhhn0", [1, G3], BF16, kind="ExternalInput")
    bhhn1 = nc.dram_tensor("bhhn1", [1, G3], BF16, kind="ExternalInput")
    out1 = nc.dram_tensor("out1", [T, PH, B], F32, kind="ExternalOutput")
    hid0 = nc.dram_tensor("hid0", [PH, B], F32, kind="ExternalOutput")
    whhT = [whh0T, whh1T]
    bhhn = [bhhn0, bhhn1]

    with tile.TileContext(nc) as tc, ExitStack() as top:
        # ---- persistent pools ----
        dram = top.enter_context(tc.tile_pool(name="dram", bufs=1,
                                              space="DRAM"))
        dram2 = top.enter_context(tc.tile_pool(name="dram2", bufs=2,
                                               space="DRAM"))
        consts = top.enter_context(tc.tile_pool(name="consts", bufs=1))
        res_pool = top.enter_context(tc.tile_pool(name="wres", bufs=1))
        stream = top.enter_context(
            tc.tile_pool(name="wstream", bufs=cfg.stream_bufs))
        ktile_pool = top.enter_context(tc.tile_pool(name="ktiles", bufs=3))
        bhh_pool = top.enter_context(tc.tile_pool(name="bhhp", bufs=1))

        # DRAM intermediates
        gi_d = [dram.tile([BT, G3], BF16, name=f"gi{l}", tag=f"gi{l}")
                for l in range(2)]

        # constants
        ident = consts.tile([64, 64], F32, name="ident", tag="ident")
        make_identity(nc, ident[:, :])
        ones_sb = consts.tile([1, 128], BF16, name="ones_sb", tag="ones_sb")
        nc.gpsimd.memset(ones_sb[:, :], 1.0)
        zrow = consts.tile([1, CS], BF16, name="zrow", tag="zrow")
        nc.gpsimd.memset(zrow[:, :], 0.0)

        def hwq(i):
            """Alternate bulk streams across the two HW-DGE queues."""
            return nc.sync if i % 2 == 0 else nc.scalar

        def load_resident(layer):
            wres = res_pool.tile([128, max(R, 1), G3], BF16, name="wres",
                                 tag="wres")
            for ko in range(R):
                hwq(ko).dma_start(wres[:, ko, :],
                                  whhT[layer][ko * 128:(ko + 1) * 128, :])
            return wres

        def gi0_gemm():
            """gi0 = [x;1]^T @ [W_ih0^T;bias] -> gi_d[0] (BT,G3) bf16."""
            with ExitStack() as ctx:
                psum = ctx.enter_context(
                    tc.tile_pool(name="gi0psum", bufs=4, space="PSUM"))
                outp = ctx.enter_context(tc.tile_pool(name="gi0out", bufs=4))
                xts, wts = [], []
                for i, (off, sz) in enumerate(cfg.in0_ks):
                    xt = ktile_pool.tile([128, BT], BF16, name=f"x{i}",
                                         tag="kx")
                    nc.sync.dma_start(xt[:sz, :], xT[off:off + sz, :])
                    xts.append(xt)
                    wt = stream.tile([128, G3], BF16, name=f"w{i}", tag="wst")
                    nc.sync.dma_start(wt[:sz, :], wih0T[off:off + sz, :])
                    wts.append(wt)
                for ch in range(NCH):
                    cs = slice(ch * CS, (ch + 1) * CS)
                    for m in range(MT):
                        ms = slice(m * 128, (m + 1) * 128)
                        pt = psum.tile([128, CS], F32, name="pt", tag="gp")
                        for i, (off, sz) in enumerate(cfg.in0_ks):
                            nc.tensor.matmul(
                                pt[:, :], xts[i][:sz, ms], wts[i][:sz, cs],
                                start=(i == 0),
                                stop=(i == len(cfg.in0_ks) - 1))
                        ot = outp.tile([128, CS], BF16, name="ot", tag="go")
                        nc.vector.tensor_copy(ot[:, :], pt[:, :])
                        nc.sync.dma_start(gi_d[0][ms, cs], ot[:, :])

        def gi1_gemm(h0_tiles):
            """gi1 = [h0;1]^T @ [W_ih1^T;bias] -> gi_d[1] (BT,G3) bf16.

            h0_tiles: layer-0's T AllGather outputs, each (128, KT, B).
            All of h0 is preloaded into SBUF once (T contiguous DMAs)."""
            with ExitStack() as ctx:
                psum = ctx.enter_context(
                    tc.tile_pool(name="gi1psum", bufs=MT, space="PSUM"))
                outp = ctx.enter_context(tc.tile_pool(name="gi1out", bufs=4))
                bias = ctx.enter_context(tc.tile_pool(name="gi1bias", bufs=1))
                bt_ = bias.tile([1, G3], BF16, name="wih1b", tag="wih1b")
                nc.sync.dma_start(bt_[:, :], wih1T[H:H + 1, :])
                # h0 staging shares the weight-residency slot (layer-0's
                # resident weights are dead here; layer-1's load comes after)
                h0sb = res_pool.tile([128, KT, T, B], BF16, name="h0sb",
                                     tag="wres")
                for t in range(T):
                    hwq(t).dma_start(h0sb[:, :, t, :], h0_tiles[t])
                for ch in range(NCH):
                    cs = slice(ch * CS, (ch + 1) * CS)
                    pts = [psum.tile([128, CS], F32, name="p1", tag="g1p")
                           for _ in range(MT)]
                    for ko in range(KT):
                        ks = slice(ko * 128, (ko + 1) * 128)
                        wt = stream.tile([128, G3], BF16, name="w1",
                                         tag="wst")
                        hwq(ko).dma_start(wt[:, :CS], wih1T[ks, cs])
                        for m in range(MT):
                            nc.tensor.matmul(
                                pts[m][:, :], h0sb[:, ko, 2 * m:2 * m + 2, :],
                                wt[:, :CS], start=(ko == 0), stop=False)
                    for m in range(MT):
                        nc.tensor.matmul(     # bias row via K=1 ones matmul
                            pts[m][:, :], ones_sb[:, :128], bt_[:, cs],
                            start=False, stop=True)
                        ot = outp.tile([128, CS], BF16, name="o1", tag="g1o")
                        nc.vector.tensor_copy(ot[:, :], pts[m][:, :])
                        nc.sync.dma_start(
                            gi_d[1][m * 128:(m + 1) * 128, cs], ot[:, :])

        def recurrence(layer, wres):
            """T GRU steps for one layer."""
            with ExitStack() as ctx:
                psum = ctx.enter_context(
                    tc.tile_pool(name="ghpsum", bufs=NCH, space="PSUM"))
                trps = ctx.enter_context(
                    tc.tile_pool(name="trpsum", bufs=2, space="PSUM"))
                hsb_p = ctx.enter_context(tc.tile_pool(name="hsb", bufs=2))
                gi_p = ctx.enter_context(tc.tile_pool(name="gis", bufs=2))
                ew = ctx.enter_context(tc.tile_pool(name="ew", bufs=1))
                hn_p = ctx.enter_context(tc.tile_pool(name="hnew", bufs=2))
                tr_p = ctx.enter_context(tc.tile_pool(name="htr", bufs=2))

                bh = bhh_pool.tile([1, G3], BF16, name="bh", tag="bh")
                nc.sync.dma_start(bh[:, :], bhhn[layer][:, :])

                h_prev = None
                h_gathered = None   # DRAM AP of last AllGather output
                ag_tiles = []       # layer-0 AG outputs (for gi1)
                # chunk ch lives in PSUM tile ch//2, partition half ch%2 —
                # pairs of chunks share a bank and run in separate PE column
                # groups concurrently.
                nbank = (NCH + 1) // 2 if coltile else NCH

                def chunk_ap(pts, ch):
                    if coltile:
                        return pts[ch // 2][(ch % 2) * 64:(ch % 2) * 64 + 64,
                                            :]
                    return pts[ch][0:64, :]

                # interleave resident and streamed K-tiles so the streamed
                # DMA demand is spread evenly across the PE block
                NS = KT - R
                if R and NS:
                    ko_order = sorted(
                        range(KT),
                        key=lambda ko: ((ko - R + 0.5) / NS) if ko >= R
                        else ((ko + 0.5) / R))
                else:
                    ko_order = list(range(KT))
                nq = [0]   # stream-queue round robin counter

                for t in range(T):
                    # ---- gh = h_{t-1} @ W_hh^T (+ bhh_n) in PSUM ----
                    # bank-outer: each gate's bank fully accumulates before
                    # the next, so its gate math overlaps later banks.
                    pts = [psum.tile([128, CS] if coltile else [64, CS], F32,
                                     name="gh", tag="gh")
                           for _ in range(nbank)]
                    np_ = 128 if coltile else 64
                    if t > 0:
                        hsb = hsb_p.tile([128, KT, B], BF16, name="hsbt",
                                         tag="hsbt")
                        nc.gpsimd.dma_start(hsb[:, :, :], h_gathered)
                    for bk in range(nbank):
                        chl = [c for c in ([2 * bk, 2 * bk + 1] if coltile
                                           else [bk]) if c < NCH]
                        c0 = chl[0] * CS
                        cw = len(chl) * CS
                        # full-bank zero seed opens the accumulation group;
                        # every later matmul overlaps it (ordering + zeros
                        # with has_written set -> clean accumulate)
                        nc.tensor.matmul(pts[bk][:np_, :],
                                         ones_sb[:, :np_], zrow[:, :],
                                         start=True, stop=False)
                        if t > 0:
                            for ko in ko_order:
                                if ko < R:
                                    wt = wres[:, ko, c0:c0 + cw]
                                else:
                                    wtile = stream.tile(
                                        [128, 2 * CS], BF16, name="ws",
                                        tag="wsl", bufs=9)
                                    nq[0] += 1
                                    hwq(nq[0]).dma_start(
                                        wtile[:, :cw],
                                        whhT[layer][ko * 128:(ko + 1) * 128,
                                                    c0:c0 + cw])
                                    wt = wtile[:, :cw]
                                for i, ch in enumerate(chl):
                                    nc.tensor.matmul(
                                        chunk_ap(pts, ch), hsb[:, ko, :],
                                        wt[:, i * CS:(i + 1) * CS],
                                        start=False, stop=False)
                        for ch in chl:
                            nc.tensor.matmul(
                                chunk_ap(pts, ch), ones_sb[:, :64],
                                bh[:, ch * CS:(ch + 1) * CS],
                                start=False, stop=False)
                        nc.tensor.matmul(pts[bk][:np_, :],
                                         ones_sb[:, :np_], zrow[:, :],
                                         start=False, stop=True)

                    # ---- elementwise gates ----
                    gi_t = gi_p.tile([64, G3], BF16, name="git", tag="git")
                    nc.sync.dma_start(
                        gi_t[:, :], gi_d[layer][t * 64:(t + 1) * 64, :])
                    s = ew.tile([64, 2 * PH], F32, name="s", tag="s")
                    rz = ew.tile([64, 2 * PH], F32, name="rz", tag="rz")
                    npre = ew.tile([64, PH], F32, name="npre", tag="npre")
                    nadd = ew.tile([64, PH], F32, name="nadd", tag="nadd")
                    nt = ew.tile([64, PH], F32, name="nt", tag="nt")
                    d = ew.tile([64, PH], F32, name="d", tag="d")
                    e = ew.tile([64, PH], F32, name="e", tag="e")
                    h_new = hn_p.tile([64, PH], F32, name="hnw", tag="hnw")
                    for ch in range(2 * NCHG):   # r and z chunks
                        cls = slice(ch * CS, (ch + 1) * CS)
                        nc.vector.tensor_add(s[:, cls], chunk_ap(pts, ch),
                                             gi_t[:, cls])
                    nc.scalar.activation(rz[:, :PH], s[:, :PH], ACT.Sigmoid)
                    nc.scalar.activation(rz[:, PH:], s[:, PH:], ACT.Sigmoid)
                    for j in range(NCHG):        # n chunks: r * gh_n
                        cls = slice(j * CS, (j + 1) * CS)
                        nc.vector.tensor_mul(npre[:, cls],
                                             chunk_ap(pts, 2 * NCHG + j),
                                             rz[:, cls])
                    nc.vector.tensor_add(nadd[:, :], npre[:, :],
                                         gi_t[:, 2 * PH:])
                    nc.scalar.activation(nt[:, :], nadd[:, :], ACT.Tanh)
                    if t == 0:
                        nc.vector.tensor_mul(e[:, :], rz[:, PH:], nt[:, :])
                        nc.vector.tensor_sub(h_new[:, :], nt[:, :], e[:, :])
                    else:
                        nc.vector.tensor_sub(d[:, :], h_prev[:, :], nt[:, :])
                        nc.vector.tensor_mul(e[:, :], rz[:, PH:], d[:, :])
                        nc.vector.tensor_add(h_new[:, :], nt[:, :], e[:, :])
                    h_prev = h_new

                    # ---- transpose h_new -> (PH,64): f32 out + bf16 AG ----
                    need_f32 = (layer == 1) or (t == T - 1)
                    hsh = tr_p.tile([128, len(cfg.tr), B], BF16, name="hsh",
                                    tag="hsh")
                    if need_f32:
                        htr = tr_p.tile([128, len(cfg.tr), B], F32,
                                        name="htf", tag="htf")
                    for j, (off, sz) in enumerate(cfg.tr):
                        tp = trps.tile([128, 64], F32, name="tp", tag="tp")
                        nc.tensor.transpose(tp[:sz, :],
                                            h_new[:, off:off + sz],
                                            ident[:, :])
                        if need_f32:
                            nc.vector.tensor_copy(htr[:sz, j, :], tp[:sz, :])
                        nc.vector.tensor_copy(hsh[:sz, j, :], tp[:sz, :])

                    if layer == 1:
                        for j, (off, sz) in enumerate(cfg.tr):
                            nc.sync.dma_start(out1[t, off:off + sz, :],
                                              htr[:sz, j, :])
                    if layer == 0 and t == T - 1:
                        for j, (off, sz) in enumerate(cfg.tr):
                            nc.sync.dma_start(hid0[off:off + sz, :],
                                              htr[:sz, j, :])

                    # ---- AllGather h across cores ----
                    if layer == 1 and t == T - 1:
                        continue
                    # local shard (PH, B) in pl-major local-unit order; the
                    # 8 shards concatenate to the packed (128, KT, B) layout.
                    hshard = dram2.tile([PH, B], BF16, name="hshd",
                                        tag="hshd")
                    for j, (off, sz) in enumerate(cfg.tr):
                        nc.gpsimd.dma_start(hshard[off:off + sz, :],
                                            hsh[:sz, j, :])
                    hc = dram2.tile([128, KT, B], BF16, name="hc", tag="hc",
                                    bufs=max(T, 2), addr_space="Shared")
                    nc.gpsimd.collective_compute(
                        "AllGather", mybir.AluOpType.bypass,
                        replica_groups=rg,
                        ins=[hshard[:, :].opt()],
                        outs=[hc[:, :, :].opt()],
                    )
                    if layer == 0:
                        ag_tiles.append(hc[:, :, :])
                    h_gathered = hc[:, :, :]
                return ag_tiles

        # ---- phase structure ----
        gi0_gemm()
        wres0 = load_resident(0)
        h0_tiles = recurrence(0, wres0)
        gi1_gemm(h0_tiles)
        wres1 = load_resident(1)
        recurrence(1, wres1)

    nc.compile()
    return nc


# --------------------------------------------------------------------------
# host side
# --------------------------------------------------------------------------

def _unit_order(cfg: Cfg, c: int) -> np.ndarray:
    """Global hidden-unit index for core c's local units 0..PH-1.
    Local order is pl-major: j = pl*KT + ko -> unit ko*128 + c*PL + pl."""
    ar = np.arange(cfg.PH)
    return (ar % cfg.KT) * 128 + c * PL + (ar // cfg.KT)


def prep_inputs(cfg: Cfg, inputs, W_ih0, W_hh0, b_ih0, b_hh0,
                W_ih1, W_hh1, b_ih1, b_hh1):
    B, T, H, PH, IN0 = cfg.B, cfg.T, cfg.H, cfg.PH, cfg.IN0
    inputs = np.asarray(inputs, np.float32)
    assert inputs.shape == (B, T, IN0)
    xT = np.empty((IN0 + 1, cfg.BT), dtype=np.float32)
    xT[:IN0] = inputs.transpose(2, 1, 0).reshape(IN0, T * B)  # col = t*B+b
    xT[IN0] = 1.0
    xT = xT.astype(BF16NP)

    arrs = {k: np.asarray(v, np.float32) for k, v in dict(
        W_ih0=W_ih0, W_hh0=W_hh0, b_ih0=b_ih0, b_hh0=b_hh0,
        W_ih1=W_ih1, W_hh1=W_hh1, b_ih1=b_ih1, b_hh1=b_hh1).items()}

    def shard(c):
        unit = _unit_order(cfg, c)
        idx = np.concatenate([g * H + unit for g in range(3)])

        def wt(W, b_i, b_h, kdim):
            out = np.empty((kdim + 1, 3 * PH), dtype=np.float32)
            out[:kdim] = W[idx].T
            bias = b_i[idx].copy()
            bias[:2 * PH] += b_h[idx][:2 * PH]   # r,z: b_ih+b_hh; n: b_ih
            out[kdim] = bias
            return out.astype(BF16NP)

        m = {
            "xT": xT,
            "wih0T": wt(arrs["W_ih0"], arrs["b_ih0"], arrs["b_hh0"], IN0),
            "wih1T": wt(arrs["W_ih1"], arrs["b_ih1"], arrs["b_hh1"], H),
            "whh0T": np.ascontiguousarray(arrs["W_hh0"][idx].T).astype(BF16NP),
            "whh1T": np.ascontiguousarray(arrs["W_hh1"][idx].T).astype(BF16NP),
        }
        for l in range(2):
            row = np.zeros((1, 3 * PH), dtype=np.float32)
            row[0, 2 * PH:] = arrs[f"b_hh{l}"][idx][2 * PH:]
            m[f"bhhn{l}"] = row.astype(BF16NP)
        return m

    return [shard(c) for c in range(NCORES)]


def assemble(cfg: Cfg, outs):
    T, B, H, KT = cfg.T, cfg.B, cfg.H, cfg.KT
    # local unit j=(pl*KT+ko) on core c -> global unit ko*128 + c*PL + pl
    out1 = np.stack([outs[c]["out1"] for c in range(NCORES)])  # (8,T,PH,B)
    out1 = out1.reshape(NCORES, T, PL, KT, B)
    output = np.ascontiguousarray(
        out1.transpose(1, 4, 3, 0, 2).reshape(T, B, H))
    hid0 = np.stack([outs[c]["hid0"] for c in range(NCORES)])  # (8,PH,B)
    hid0 = hid0.reshape(NCORES, PL, KT, B)
    h0_last = hid0.transpose(3, 2, 0, 1).reshape(B, H)
    hidden = np.stack([h0_last, output[T - 1]], axis=0)
    return output.astype(np.float32), hidden.astype(np.float32)


LAST_RESULTS = None


def _ensure_ntff_hook():
    """The agent image's antenv lacks axon_hooks; recreate it so
    trace=True can drive NTFF profiling via the injected libaxon so."""
    try:
        from antenv.axon_hooks import get_axon_ntff_profile_hook  # noqa: F401
        return
    except ImportError:
        pass
    import sys
    import types

    import antenv

    mod = types.ModuleType("antenv.axon_hooks")
    _hook = [None]
    mod.set_axon_ntff_profile_hook = lambda h: _hook.__setitem__(0, h)
    mod.get_axon_ntff_profile_hook = lambda: _hook[0]
    sys.modules["antenv.axon_hooks"] = mod
    antenv.axon_hooks = mod
    try:
        if "/root/.axon_site" not in sys.path:
            sys.path.insert(0, "/root/.axon_site")
        from trn_agent_boot.trn_boot import _ntff_profile_via_ctypes
        so = "/opt/axon/libaxon_pjrt.so"
        if os.path.exists(so):
            mod.set_axon_ntff_profile_hook(_ntff_profile_via_ctypes(so))
    except Exception:
        pass


def kernel(inputs, W_ih0, W_hh0, b_ih0, b_hh0, W_ih1, W_hh1, b_ih1, b_hh1):
    global LAST_RESULTS
    cfg = Cfg(R=int(os.environ.get("GRU_R", "21")),
              stream_bufs=int(os.environ.get("GRU_SB", "4")))
    nc = build_nc(cfg)
    in_maps = prep_inputs(cfg, inputs, W_ih0, W_hh0, b_ih0, b_hh0,
                          W_ih1, W_hh1, b_ih1, b_hh1)
    trace = bool(int(os.environ.get("GRU_TRACE", "0")))
    if trace:
        _ensure_ntff_hook()
    res = run_bass_kernel_spmd(nc, in_maps, list(range(NCORES)), trace=trace)
    LAST_RESULTS = res
    return assemble(cfg, res.results)


# revision 36
# speedup vs baseline: 1.4800x; 1.0305x over previous
"""Trainium2 Bass kernel for nn_EncoderModel (2-layer GRU encoder).

Model: B=64, T=12, H=6400 (3H=19200 gate rows), IN0=200, 2 layers.

Sharding: tensor-parallel over the gate/output dim with an interleaved
unit map: core c owns hidden units {ko*128 + c*16 + pl} (16 partition rows
of every 128-row K-tile), for each of the r/z/n gate blocks. Each step every
core computes its 800-unit slice of the new hidden state; the AllGather of
the 8 per-core (16, KT, B) shards then reconstructs the full h directly in
the packed (128 partitions, KT, B) SBUF lhsT layout — one contiguous DMA.

Per-core compute layouts:
  - hh GEMM: out = h @ W^T accumulated in PSUM as (batch=64 part, gate free);
    lhsT = h^T K-tiles (128,64) stationary, rhs = W^T K-tiles (128,CS) moving.
    PE column-tiling 2x: even K-tiles accumulate into PSUM partitions 0:64,
    odd K-tiles into 64:128 (concurrent in separate PE column groups); the
    two halves are summed by the DVE at gate-evaluation time.
  - W^T (H, 3*PH) bf16 is streamed from HBM each step; first R K-tiles stay
    resident in SBUF.
  - h_new (64,PH) f32 -> PE-transposed to (PH,64), cast bf16 -> AllGather.
  - All biases are folded into the gi GEMM (ones row in lhsT, bias row in
    W_ih^T), except b_hh[n-gate], which joins gh_n via a K=1 ones-row matmul
    so n = tanh(gi_n + r*(h@W_hh_n^T + b_hh_n)) matches the reference.
"""

import os
from contextlib import ExitStack

import ml_dtypes
import numpy as np

import concourse.bass as bass
import concourse.mybir as mybir
import concourse.tile as tile
from concourse import bacc
from concourse.bass_utils import run_bass_kernel_spmd
from concourse.masks import make_identity

F32 = mybir.dt.float32
BF16 = mybir.dt.bfloat16
BF16NP = ml_dtypes.bfloat16
ACT = mybir.ActivationFunctionType

NCORES = 8
PL = 128 // NCORES   # partition rows per core within each K-tile


class Cfg:
    def __init__(self, B=64, T=12, H=6400, IN0=200, R=16, stream_bufs=5):
        self.B, self.T, self.H, self.IN0 = B, T, H, IN0
        assert B == 64
        assert H % 128 == 0
        self.PH = H // NCORES            # hidden units per core
        self.G3 = 3 * self.PH            # gate rows per core
        self.KT = H // 128               # K tiles over H
        assert self.PH == self.KT * PL
        self.CS = 400 if self.PH % 400 == 0 else self.PH   # psum chunk size
        assert self.PH % self.CS == 0 and self.CS <= 512
        self.NCHG = self.PH // self.CS   # chunks per gate
        self.NCH = 3 * self.NCHG         # psum chunks per step
        assert self.NCH + 2 <= 8, "psum banks"
        self.BT = B * T
        assert self.BT % 128 == 0
        self.MT = self.BT // 128         # m tiles for gi GEMMs
        self.R = min(R, self.KT)         # resident W K-tiles
        self.stream_bufs = stream_bufs
        # K tiling for IN0+1 (ones/bias row folded in)
        k, off, self.in0_ks = IN0 + 1, 0, []
        while off < k:
            s = min(128, k - off)
            self.in0_ks.append((off, s))
            off += s
        # transpose tiles over PH (each tile covers sz//PL K-tiles)
        off, self.tr = 0, []
        while off < self.PH:
            s = min(128, self.PH - off)
            assert s % PL == 0
            self.tr.append((off, s))
            off += s


def build_nc(cfg: Cfg) -> bass.Bass:
    B, T, H, PH, G3 = cfg.B, cfg.T, cfg.H, cfg.PH, cfg.G3
    KT, CS, NCH, NCHG, MT, R = cfg.KT, cfg.CS, cfg.NCH, cfg.NCHG, cfg.MT, cfg.R
    BT = cfg.BT
    coltile = KT >= 2
    last_odd = KT - 1 if (KT - 1) % 2 == 1 else KT - 2

    nc = bacc.Bacc("TRN2", target_bir_lowering=False, debug=False,
                   num_devices=NCORES)
    rg = [list(range(NCORES))]

    # ---- kernel I/O (per-core data via in_maps) ----
    xT = nc.dram_tensor("xT", [cfg.IN0 + 1, BT], BF16, kind="ExternalInput")
    wih0T = nc.dram_tensor("wih0T", [cfg.IN0 + 1, G3], BF16,
                           kind="ExternalInput")
    wih1T = nc.dram_tensor("wih1T", [H + 1, G3], BF16, kind="ExternalInput")
    whh0T = nc.dram_tensor("whh0T", [H, G3], BF16, kind="ExternalInput")
    whh1T = nc.dram_tensor("whh1T", [H, G3], BF16, kind="ExternalInput")
    bhhn0 = nc.dram_tensor("bhhn0", [1, G3], BF16, kind="ExternalInput")
    bhhn1 = nc.dram_tensor("bhhn1", [1, G3], BF16, kind="ExternalInput")
    out1 = nc.dram_tensor("out1", [T, PH, B], F32, kind="ExternalOutput")
    hid0 = nc.dram_tensor("hid0", [PH, B], F32, kind="ExternalOutput")
    whhT = [whh0T, whh1T]
    bhhn = [bhhn0, bhhn1]

    with tile.TileContext(nc) as tc, ExitStack() as top:
        # ---- persistent pools ----
        dram = top.enter_context(tc.tile_pool(name="dram", bufs=1,
                                              space="DRAM"))
        dram2 = top.enter_context(tc.tile_pool(name="dram2", bufs=2,
                                               space="DRAM"))
        consts = top.enter_context(tc.tile_pool(name="consts", bufs=1))
        res_pool = top.enter_context(tc.tile_pool(name="wres", bufs=1))
        stream = top.enter_context(
            tc.tile_pool(name="wstream", bufs=cfg.stream_bufs))
        ktile_pool = top.enter_context(tc.tile_pool(name="ktiles", bufs=3))
        bhh_pool = top.enter_context(tc.tile_pool(name="bhhp", bufs=1))

        # DRAM intermediates
        gi_d = [dram.tile([BT, G3], BF16, name=f"gi{l}", tag=f"gi{l}")
                for l in range(2)]

        # constants
        ident = consts.tile([64, 64], F32, name="ident", tag="ident")
        make_identity(nc, ident[:, :])
        ones_sb = consts.tile([1, 128], BF16, name="ones_sb", tag="ones_sb")
        nc.gpsimd.memset(ones_sb[:, :], 1.0)
        zrow = consts.tile([1, CS], BF16, name="zrow", tag="zrow")
        nc.gpsimd.memset(zrow[:, :], 0.0)

        def hwq(i):
            """Alternate bulk streams across the two HW-DGE queues."""
            return nc.sync if i % 2 == 0 else nc.scalar

        def load_resident(layer):
            wres = res_pool.tile([128, max(R, 1), G3], BF16, name="wres",
                                 tag="wres")
            for ko in range(R):
                hwq(ko).dma_start(wres[:, ko, :],
                                  whhT[layer][ko * 128:(ko + 1) * 128, :])
            return wres

        def gi0_gemm():
            """gi0 = [x;1]^T @ [W_ih0^T;bias] -> gi_d[0] (BT,G3) bf16."""
            with ExitStack() as ctx:
                psum = ctx.enter_context(
                    tc.tile_pool(name="gi0psum", bufs=4, space="PSUM"))
                outp = ctx.enter_context(tc.tile_pool(name="gi0out", bufs=4))
                xts, wts = [], []
                for i, (off, sz) in enumerate(cfg.in0_ks):
                    xt = ktile_pool.tile([128, BT], BF16, name=f"x{i}",
                                         tag="kx")
                    nc.sync.dma_start(xt[:sz, :], xT[off:off + sz, :])
                    xts.append(xt)
                    wt = stream.tile([128, G3], BF16, name=f"w{i}", tag="wst")
                    nc.sync.dma_start(wt[:sz, :], wih0T[off:off + sz, :])
                    wts.append(wt)
                for ch in range(NCH):
                    cs = slice(ch * CS, (ch + 1) * CS)
                    for m in range(MT):
                        ms = slice(m * 128, (m + 1) * 128)
                        pt = psum.tile([128, CS], F32, name="pt", tag="gp")
                        for i, (off, sz) in enumerate(cfg.in0_ks):
                            nc.tensor.matmul(
                                pt[:, :], xts[i][:sz, ms], wts[i][:sz, cs],
                                start=(i == 0),
                                stop=(i == len(cfg.in0_ks) - 1))
                        ot = outp.tile([128, CS], BF16, name="ot", tag="go")
                        nc.vector.tensor_copy(ot[:, :], pt[:, :])
                        nc.sync.dma_start(gi_d[0][ms, cs], ot[:, :])

        def gi1_gemm(h0_tiles):
            """gi1 = [h0;1]^T @ [W_ih1^T;bias] -> gi_d[1] (BT,G3) bf16.

            h0_tiles: layer-0's T AllGather outputs, each (128, KT, B).
            All of h0 is preloaded into SBUF once (T contiguous DMAs)."""
            with ExitStack() as ctx:
                psum = ctx.enter_context(
                    tc.tile_pool(name="gi1psum", bufs=MT, space="PSUM"))
                outp = ctx.enter_context(tc.tile_pool(name="gi1out", bufs=4))
                bias = ctx.enter_context(tc.tile_pool(name="gi1bias", bufs=1))
                bt_ = bias.tile([1, G3], BF16, name="wih1b", tag="wih1b")
                nc.sync.dma_start(bt_[:, :], wih1T[H:H + 1, :])
                # h0 staging shares the weight-residency slot (layer-0's
                # resident weights are dead here; layer-1's load comes after)
                h0sb = res_pool.tile([128, KT, T, B], BF16, name="h0sb",
                                     tag="wres")
                for t in range(T):
                    hwq(t).dma_start(h0sb[:, :, t, :], h0_tiles[t])
                for ch in range(NCH):
                    cs = slice(ch * CS, (ch + 1) * CS)
                    pts = [psum.tile([128, CS], F32, name="p1", tag="g1p")
                           for _ in range(MT)]
                    for ko in range(KT):
                        ks = slice(ko * 128, (ko + 1) * 128)
                        wt = stream.tile([128, G3], BF16, name="w1",
                                         tag="wst")
                        hwq(ko).dma_start(wt[:, :CS], wih1T[ks, cs])
                        for m in range(MT):
                            nc.tensor.matmul(
                                pts[m][:, :], h0sb[:, ko, 2 * m:2 * m + 2, :],
                                wt[:, :CS], start=(ko == 0), stop=False)
                    for m in range(MT):
                        nc.tensor.matmul(     # bias row via K=1 ones matmul
                            pts[m][:, :], ones_sb[:, :128], bt_[:, cs],
                            start=False, stop=True)
                        ot = outp.tile([128, CS], BF16, name="o1", tag="g1o")
                        nc.vector.tensor_copy(ot[:, :], pts[m][:, :])
                        nc.sync.dma_start(
                            gi_d[1][m * 128:(m + 1) * 128, cs], ot[:, :])

        def recurrence(layer, wres):
            """T GRU steps for one layer."""
            with ExitStack() as ctx:
                psum = ctx.enter_context(
                    tc.tile_pool(name="ghpsum", bufs=NCH, space="PSUM"))
                trps = ctx.enter_context(
                    tc.tile_pool(name="trpsum", bufs=2, space="PSUM"))
                hsb_p = ctx.enter_context(tc.tile_pool(name="hsb", bufs=2))
                gi_p = ctx.enter_context(tc.tile_pool(name="gis", bufs=2))
                ew = ctx.enter_context(tc.tile_pool(name="ew", bufs=1))
                hn_p = ctx.enter_context(tc.tile_pool(name="hnew", bufs=2))
                tr_p = ctx.enter_context(tc.tile_pool(name="htr", bufs=2))

                bh = bhh_pool.tile([1, G3], BF16, name="bh", tag="bh")
                nc.gpsimd.dma_start(bh[:, :], bhhn[layer][:, :])

                h_prev = None
                h_gathered = None   # DRAM AP of last AllGather output
                ag_tiles = []       # layer-0 AG outputs (for gi1)
                # chunk ch lives in PSUM tile ch//2, partition half ch%2 —
                # pairs of chunks share a bank and run in separate PE column
                # groups concurrently.
                nbank = (NCH + 1) // 2 if coltile else NCH

                def chunk_ap(pts, ch):
                    if coltile:
                        return pts[ch // 2][(ch % 2) * 64:(ch % 2) * 64 + 64,
                                            :]
                    return pts[ch][0:64, :]

                # interleave resident and streamed K-tiles so the streamed
                # DMA demand is spread evenly across the PE block
                NS = KT - R
                if R and NS:
                    ko_order = sorted(
                        range(KT),
                        key=lambda ko: ((ko - R + 0.5) / NS) if ko >= R
                        else ((ko + 0.5) / R))
                else:
                    ko_order = list(range(KT))
                nq = [0]   # stream-queue round robin counter

                for t in range(T):
                    # ---- gh = h_{t-1} @ W_hh^T (+ bhh_n) in PSUM ----
                    # bank-outer: each gate's bank fully accumulates before
                    # the next, so its gate math overlaps later banks.
                    pts = [psum.tile([128, CS] if coltile else [64, CS], F32,
                                     name="gh", tag="gh")
                           for _ in range(nbank)]
                    np_ = 128 if coltile else 64
                    if t > 0:
                        hsb = hsb_p.tile([128, KT, B], BF16, name="hsbt",
                                         tag="hsbt")
                        nc.gpsimd.dma_start(hsb[:, :, :], h_gathered)
                    for bk in range(nbank):
                        chl = [c for c in ([2 * bk, 2 * bk + 1] if coltile
                                           else [bk]) if c < NCH]
                        c0 = chl[0] * CS
                        cw = len(chl) * CS
                        # full-bank zero seed opens the accumulation group;
                        # every later matmul overlaps it (ordering + zeros
                        # with has_written set -> clean accumulate)
                        nc.tensor.matmul(pts[bk][:np_, :],
                                         ones_sb[:, :np_], zrow[:, :],
                                         start=True, stop=False)
                        if t > 0:
                            for ko in ko_order:
                                if ko < R:
                                    wt = wres[:, ko, c0:c0 + cw]
                                else:
                                    wtile = stream.tile(
                                        [128, 2 * CS], BF16, name="ws",
                                        tag="wsl", bufs=12)
                                    nq[0] += 1
                                    hwq(nq[0]).dma_start(
                                        wtile[:, :cw],
                                        whhT[layer][ko * 128:(ko + 1) * 128,
                                                    c0:c0 + cw])
                                    wt = wtile[:, :cw]
                                for i, ch in enumerate(chl):
                                    nc.tensor.matmul(
                                        chunk_ap(pts, ch), hsb[:, ko, :],
                                        wt[:, i * CS:(i + 1) * CS],
                                        start=False, stop=False)
                        for ch in chl:
                            nc.tensor.matmul(
                                chunk_ap(pts, ch), ones_sb[:, :64],
                                bh[:, ch * CS:(ch + 1) * CS],
                                start=False, stop=False)
                        nc.tensor.matmul(pts[bk][:np_, :],
                                         ones_sb[:, :np_], zrow[:, :],
                                         start=False, stop=True)

                    # ---- elementwise gates ----
                    gi_t = gi_p.tile([64, G3], BF16, name="git", tag="git")
                    nc.gpsimd.dma_start(
                        gi_t[:, :], gi_d[layer][t * 64:(t + 1) * 64, :])
                    s = ew.tile([64, 2 * PH], F32, name="s", tag="s")
                    rz = ew.tile([64, 2 * PH], F32, name="rz", tag="rz")
                    npre = ew.tile([64, PH], F32, name="npre", tag="npre")
                    nadd = ew.tile([64, PH], F32, name="nadd", tag="nadd")
                    nt = ew.tile([64, PH], F32, name="nt", tag="nt")
                    d = ew.tile([64, PH], F32, name="d", tag="d")
                    e = ew.tile([64, PH], F32, name="e", tag="e")
                    h_new = hn_p.tile([64, PH], F32, name="hnw", tag="hnw")
                    for ch in range(2 * NCHG):   # r and z chunks
                        cls = slice(ch * CS, (ch + 1) * CS)
                        nc.vector.tensor_add(s[:, cls], chunk_ap(pts, ch),
                                             gi_t[:, cls])
                    nc.scalar.activation(rz[:, :PH], s[:, :PH], ACT.Sigmoid)
                    nc.scalar.activation(rz[:, PH:], s[:, PH:], ACT.Sigmoid)
                    for j in range(NCHG):        # n chunks: r * gh_n
                        cls = slice(j * CS, (j + 1) * CS)
                        nc.vector.tensor_mul(npre[:, cls],
                                             chunk_ap(pts, 2 * NCHG + j),
                                             rz[:, cls])
                        nc.vector.tensor_add(nadd[:, cls], npre[:, cls],
                                             gi_t[:, 2 * PH + j * CS:
                                                  2 * PH + (j + 1) * CS])
                        nc.scalar.activation(nt[:, cls], nadd[:, cls],
                                             ACT.Tanh)
                        if t == 0:
                            nc.vector.tensor_mul(e[:, cls], rz[:, PH:][:, cls],
                                                 nt[:, cls])
                            nc.vector.tensor_sub(h_new[:, cls], nt[:, cls],
                                                 e[:, cls])
                        else:
                            nc.vector.tensor_sub(d[:, cls],
                                                 h_prev[:, cls],
                                                 nt[:, cls])
                            nc.vector.tensor_mul(e[:, cls], rz[:, PH:][:, cls],
                                                 d[:, cls])
                            nc.vector.tensor_add(h_new[:, cls], nt[:, cls],
                                                 e[:, cls])
                    h_prev = h_new

                    # ---- transpose h_new -> (PH,64): f32 out + bf16 AG ----
                    need_f32 = (layer == 1) or (t == T - 1)
                    hsh = tr_p.tile([128, len(cfg.tr), B], BF16, name="hsh",
                                    tag="hsh")
                    if need_f32:
                        htr = tr_p.tile([128, len(cfg.tr), B], F32,
                                        name="htf", tag="htf")
                    for j, (off, sz) in enumerate(cfg.tr):
                        tp = trps.tile([128, 64], F32, name="tp", tag="tp")
                        nc.tensor.transpose(tp[:sz, :],
                                            h_new[:, off:off + sz],
                                            ident[:, :])
                        if need_f32:
                            nc.vector.tensor_copy(htr[:sz, j, :], tp[:sz, :])
                        nc.vector.tensor_copy(hsh[:sz, j, :], tp[:sz, :])

                    if layer == 1:
                        for j, (off, sz) in enumerate(cfg.tr):
                            nc.sync.dma_start(out1[t, off:off + sz, :],
                                              htr[:sz, j, :])
                    if layer == 0 and t == T - 1:
                        for j, (off, sz) in enumerate(cfg.tr):
                            nc.sync.dma_start(hid0[off:off + sz, :],
                                              htr[:sz, j, :])

                    # ---- AllGather h across cores ----
                    if layer == 1 and t == T - 1:
                        continue
                    # local shard (PH, B) in pl-major local-unit order; the
                    # 8 shards concatenate to the packed (128, KT, B) layout.
                    hshard = dram2.tile([PH, B], BF16, name="hshd",
                                        tag="hshd")
                    for j, (off, sz) in enumerate(cfg.tr):
                        nc.gpsimd.dma_start(hshard[off:off + sz, :],
                                            hsh[:sz, j, :])
                    hc = dram2.tile([128, KT, B], BF16, name="hc", tag="hc",
                                    bufs=max(T, 2), addr_space="Shared")
                    nc.gpsimd.collective_compute(
                        "AllGather", mybir.AluOpType.bypass,
                        replica_groups=rg,
                        ins=[hshard[:, :].opt()],
                        outs=[hc[:, :, :].opt()],
                    )
                    if layer == 0:
                        ag_tiles.append(hc[:, :, :])
                    h_gathered = hc[:, :, :]
                return ag_tiles

        # ---- phase structure ----
        gi0_gemm()
        wres0 = load_resident(0)
        h0_tiles = recurrence(0, wres0)
        gi1_gemm(h0_tiles)
        wres1 = load_resident(1)
        recurrence(1, wres1)

    nc.compile()
    return nc


# --------------------------------------------------------------------------
# host side
# --------------------------------------------------------------------------

def _unit_order(cfg: Cfg, c: int) -> np.ndarray:
    """Global hidden-unit index for core c's local units 0..PH-1.
    Local order is pl-major: j = pl*KT + ko -> unit ko*128 + c*PL + pl."""
    ar = np.arange(cfg.PH)
    return (ar % cfg.KT) * 128 + c * PL + (ar // cfg.KT)


def prep_inputs(cfg: Cfg, inputs, W_ih0, W_hh0, b_ih0, b_hh0,
                W_ih1, W_hh1, b_ih1, b_hh1):
    B, T, H, PH, IN0 = cfg.B, cfg.T, cfg.H, cfg.PH, cfg.IN0
    inputs = np.asarray(inputs, np.float32)
    assert inputs.shape == (B, T, IN0)
    xT = np.empty((IN0 + 1, cfg.BT), dtype=np.float32)
    xT[:IN0] = inputs.transpose(2, 1, 0).reshape(IN0, T * B)  # col = t*B+b
    xT[IN0] = 1.0
    xT = xT.astype(BF16NP)

    arrs = {k: np.asarray(v, np.float32) for k, v in dict(
        W_ih0=W_ih0, W_hh0=W_hh0, b_ih0=b_ih0, b_hh0=b_hh0,
        W_ih1=W_ih1, W_hh1=W_hh1, b_ih1=b_ih1, b_hh1=b_hh1).items()}

    def shard(c):
        unit = _unit_order(cfg, c)
        idx = np.concatenate([g * H + unit for g in range(3)])

        def wt(W, b_i, b_h, kdim):
            out = np.empty((kdim + 1, 3 * PH), dtype=np.float32)
            out[:kdim] = W[idx].T
            bias = b_i[idx].copy()
            bias[:2 * PH] += b_h[idx][:2 * PH]   # r,z: b_ih+b_hh; n: b_ih
            out[kdim] = bias
            return out.astype(BF16NP)

        m = {
            "xT": xT,
            "wih0T": wt(arrs["W_ih0"], arrs["b_ih0"], arrs["b_hh0"], IN0),
            "wih1T": wt(arrs["W_ih1"], arrs["b_ih1"], arrs["b_hh1"], H),
            "whh0T": np.ascontiguousarray(arrs["W_hh0"][idx].T).astype(BF16NP),
            "whh1T": np.ascontiguousarray(arrs["W_hh1"][idx].T).astype(BF16NP),
        }
        for l in range(2):
            row = np.zeros((1, 3 * PH), dtype=np.float32)
            row[0, 2 * PH:] = arrs[f"b_hh{l}"][idx][2 * PH:]
            m[f"bhhn{l}"] = row.astype(BF16NP)
        return m

    return [shard(c) for c in range(NCORES)]


def assemble(cfg: Cfg, outs):
    T, B, H, KT = cfg.T, cfg.B, cfg.H, cfg.KT
    # local unit j=(pl*KT+ko) on core c -> global unit ko*128 + c*PL + pl
    out1 = np.stack([outs[c]["out1"] for c in range(NCORES)])  # (8,T,PH,B)
    out1 = out1.reshape(NCORES, T, PL, KT, B)
    output = np.ascontiguousarray(
        out1.transpose(1, 4, 3, 0, 2).reshape(T, B, H))
    hid0 = np.stack([outs[c]["hid0"] for c in range(NCORES)])  # (8,PH,B)
    hid0 = hid0.reshape(NCORES, PL, KT, B)
    h0_last = hid0.transpose(3, 2, 0, 1).reshape(B, H)
    hidden = np.stack([h0_last, output[T - 1]], axis=0)
    return output.astype(np.float32), hidden.astype(np.float32)


LAST_RESULTS = None


def _ensure_ntff_hook():
    """The agent image's antenv lacks axon_hooks; recreate it so
    trace=True can drive NTFF profiling via the injected libaxon so."""
    try:
        from antenv.axon_hooks import get_axon_ntff_profile_hook  # noqa: F401
        return
    except ImportError:
        pass
    import sys
    import types

    import antenv

    mod = types.ModuleType("antenv.axon_hooks")
    _hook = [None]
    mod.set_axon_ntff_profile_hook = lambda h: _hook.__setitem__(0, h)
    mod.get_axon_ntff_profile_hook = lambda: _hook[0]
    sys.modules["antenv.axon_hooks"] = mod
    antenv.axon_hooks = mod
    try:
        if "/root/.axon_site" not in sys.path:
            sys.path.insert(0, "/root/.axon_site")
        from trn_agent_boot.trn_boot import _ntff_profile_via_ctypes
        so = "/opt/axon/libaxon_pjrt.so"
        if os.path.exists(so):
            mod.set_axon_ntff_profile_hook(_ntff_profile_via_ctypes(so))
    except Exception:
        pass


def kernel(inputs, W_ih0, W_hh0, b_ih0, b_hh0, W_ih1, W_hh1, b_ih1, b_hh1):
    global LAST_RESULTS
    cfg = Cfg(R=int(os.environ.get("GRU_R", "20")),
              stream_bufs=int(os.environ.get("GRU_SB", "4")))
    nc = build_nc(cfg)
    in_maps = prep_inputs(cfg, inputs, W_ih0, W_hh0, b_ih0, b_hh0,
                          W_ih1, W_hh1, b_ih1, b_hh1)
    trace = bool(int(os.environ.get("GRU_TRACE", "0")))
    if trace:
        _ensure_ntff_hook()
    res = run_bass_kernel_spmd(nc, in_maps, list(range(NCORES)), trace=trace)
    LAST_RESULTS = res
    return assemble(cfg, res.results)


# revision 37
# speedup vs baseline: 1.4802x; 1.0001x over previous
"""Trainium2 Bass kernel for nn_EncoderModel (2-layer GRU encoder).

Model: B=64, T=12, H=6400 (3H=19200 gate rows), IN0=200, 2 layers.

Sharding: tensor-parallel over the gate/output dim with an interleaved
unit map: core c owns hidden units {ko*128 + c*16 + pl} (16 partition rows
of every 128-row K-tile), for each of the r/z/n gate blocks. Each step every
core computes its 800-unit slice of the new hidden state; the AllGather of
the 8 per-core (16, KT, B) shards then reconstructs the full h directly in
the packed (128 partitions, KT, B) SBUF lhsT layout — one contiguous DMA.

Per-core compute layouts:
  - hh GEMM: out = h @ W^T accumulated in PSUM as (batch=64 part, gate free);
    lhsT = h^T K-tiles (128,64) stationary, rhs = W^T K-tiles (128,CS) moving.
    PE column-tiling 2x: even K-tiles accumulate into PSUM partitions 0:64,
    odd K-tiles into 64:128 (concurrent in separate PE column groups); the
    two halves are summed by the DVE at gate-evaluation time.
  - W^T (H, 3*PH) bf16 is streamed from HBM each step; first R K-tiles stay
    resident in SBUF.
  - h_new (64,PH) f32 -> PE-transposed to (PH,64), cast bf16 -> AllGather.
  - All biases are folded into the gi GEMM (ones row in lhsT, bias row in
    W_ih^T), except b_hh[n-gate], which joins gh_n via a K=1 ones-row matmul
    so n = tanh(gi_n + r*(h@W_hh_n^T + b_hh_n)) matches the reference.
"""

import os
from contextlib import ExitStack

import ml_dtypes
import numpy as np

import concourse.bass as bass
import concourse.mybir as mybir
import concourse.tile as tile
from concourse import bacc
from concourse.bass_utils import run_bass_kernel_spmd
from concourse.masks import make_identity

F32 = mybir.dt.float32
BF16 = mybir.dt.bfloat16
BF16NP = ml_dtypes.bfloat16
ACT = mybir.ActivationFunctionType

NCORES = 8
PL = 128 // NCORES   # partition rows per core within each K-tile


class Cfg:
    def __init__(self, B=64, T=12, H=6400, IN0=200, R=16, stream_bufs=5):
        self.B, self.T, self.H, self.IN0 = B, T, H, IN0
        assert B == 64
        assert H % 128 == 0
        self.PH = H // NCORES            # hidden units per core
        self.G3 = 3 * self.PH            # gate rows per core
        self.KT = H // 128               # K tiles over H
        assert self.PH == self.KT * PL
        self.CS = 400 if self.PH % 400 == 0 else self.PH   # psum chunk size
        assert self.PH % self.CS == 0 and self.CS <= 512
        self.NCHG = self.PH // self.CS   # chunks per gate
        self.NCH = 3 * self.NCHG         # psum chunks per step
        assert self.NCH + 2 <= 8, "psum banks"
        self.BT = B * T
        assert self.BT % 128 == 0
        self.MT = self.BT // 128         # m tiles for gi GEMMs
        self.R = min(R, self.KT)         # resident W K-tiles
        self.stream_bufs = stream_bufs
        # K tiling for IN0+1 (ones/bias row folded in)
        k, off, self.in0_ks = IN0 + 1, 0, []
        while off < k:
            s = min(128, k - off)
            self.in0_ks.append((off, s))
            off += s
        # transpose tiles over PH (each tile covers sz//PL K-tiles)
        off, self.tr = 0, []
        while off < self.PH:
            s = min(128, self.PH - off)
            assert s % PL == 0
            self.tr.append((off, s))
            off += s


def build_nc(cfg: Cfg) -> bass.Bass:
    B, T, H, PH, G3 = cfg.B, cfg.T, cfg.H, cfg.PH, cfg.G3
    KT, CS, NCH, NCHG, MT, R = cfg.KT, cfg.CS, cfg.NCH, cfg.NCHG, cfg.MT, cfg.R
    BT = cfg.BT
    coltile = KT >= 2

    nc = bacc.Bacc("TRN2", target_bir_lowering=False, debug=False,
                   num_devices=NCORES)
    rg = [list(range(NCORES))]

    # ---- kernel I/O (per-core data via in_maps) ----
    xT = nc.dram_tensor("xT", [cfg.IN0 + 1, BT], BF16, kind="ExternalInput")
    wih0T = nc.dram_tensor("wih0T", [cfg.IN0 + 1, G3], BF16,
                           kind="ExternalInput")
    wih1T = nc.dram_tensor("wih1T", [H + 1, G3], BF16, kind="ExternalInput")
    whh0T = nc.dram_tensor("whh0T", [H, G3], BF16, kind="ExternalInput")
    whh1T = nc.dram_tensor("whh1T", [H, G3], BF16, kind="ExternalInput")
    bhhn0 = nc.dram_tensor("bhhn0", [1, G3], BF16, kind="ExternalInput")
    bhhn1 = nc.dram_tensor("bhhn1", [1, G3], BF16, kind="ExternalInput")
    out1 = nc.dram_tensor("out1", [T, PH, B], F32, kind="ExternalOutput")
    hid0 = nc.dram_tensor("hid0", [PH, B], F32, kind="ExternalOutput")
    whhT = [whh0T, whh1T]
    bhhn = [bhhn0, bhhn1]

    with tile.TileContext(nc) as tc, ExitStack() as top:
        # ---- persistent pools ----
        dram = top.enter_context(tc.tile_pool(name="dram", bufs=1,
                                              space="DRAM"))
        dram2 = top.enter_context(tc.tile_pool(name="dram2", bufs=2,
                                               space="DRAM"))
        consts = top.enter_context(tc.tile_pool(name="consts", bufs=1))
        res_pool = top.enter_context(tc.tile_pool(name="wres", bufs=1))
        stream = top.enter_context(
            tc.tile_pool(name="wstream", bufs=cfg.stream_bufs))
        ktile_pool = top.enter_context(tc.tile_pool(name="ktiles", bufs=3))
        bhh_pool = top.enter_context(tc.tile_pool(name="bhhp", bufs=1))

        # DRAM intermediates
        gi_d = [dram.tile([BT, G3], BF16, name=f"gi{l}", tag=f"gi{l}")
                for l in range(2)]

        # constants
        ident = consts.tile([64, 64], F32, name="ident", tag="ident")
        make_identity(nc, ident[:, :])
        ones_sb = consts.tile([1, 128], BF16, name="ones_sb", tag="ones_sb")
        nc.gpsimd.memset(ones_sb[:, :], 1.0)
        zrow = consts.tile([1, CS], BF16, name="zrow", tag="zrow")
        nc.gpsimd.memset(zrow[:, :], 0.0)

        def hwq(i):
            """Alternate bulk streams across the two HW-DGE queues."""
            return nc.sync if i % 2 == 0 else nc.scalar

        def load_resident(layer):
            wres = res_pool.tile([128, max(R, 1), G3], BF16, name="wres",
                                 tag="wres")
            for ko in range(R):
                hwq(ko).dma_start(wres[:, ko, :],
                                  whhT[layer][ko * 128:(ko + 1) * 128, :])
            return wres

        def gi0_gemm():
            """gi0 = [x;1]^T @ [W_ih0^T;bias] -> gi_d[0] (BT,G3) bf16."""
            with ExitStack() as ctx:
                psum = ctx.enter_context(
                    tc.tile_pool(name="gi0psum", bufs=4, space="PSUM"))
                outp = ctx.enter_context(tc.tile_pool(name="gi0out", bufs=4))
                xts, wts = [], []
                for i, (off, sz) in enumerate(cfg.in0_ks):
                    xt = ktile_pool.tile([128, BT], BF16, name=f"x{i}",
                                         tag="kx")
                    nc.sync.dma_start(xt[:sz, :], xT[off:off + sz, :])
                    xts.append(xt)
                    wt = stream.tile([128, G3], BF16, name=f"w{i}", tag="wst")
                    nc.sync.dma_start(wt[:sz, :], wih0T[off:off + sz, :])
                    wts.append(wt)
                for ch in range(NCH):
                    cs = slice(ch * CS, (ch + 1) * CS)
                    for m in range(MT):
                        ms = slice(m * 128, (m + 1) * 128)
                        pt = psum.tile([128, CS], F32, name="pt", tag="gp")
                        for i, (off, sz) in enumerate(cfg.in0_ks):
                            nc.tensor.matmul(
                                pt[:, :], xts[i][:sz, ms], wts[i][:sz, cs],
                                start=(i == 0),
                                stop=(i == len(cfg.in0_ks) - 1))
                        ot = outp.tile([128, CS], BF16, name="ot", tag="go")
                        nc.vector.tensor_copy(ot[:, :], pt[:, :])
                        nc.sync.dma_start(gi_d[0][ms, cs], ot[:, :])

        def gi1_gemm(h0_tiles):
            """gi1 = [h0;1]^T @ [W_ih1^T;bias] -> gi_d[1] (BT,G3) bf16.

            h0_tiles: layer-0's T AllGather outputs, each (128, KT, B).
            All of h0 is preloaded into SBUF once (T contiguous DMAs)."""
            with ExitStack() as ctx:
                psum = ctx.enter_context(
                    tc.tile_pool(name="gi1psum", bufs=MT, space="PSUM"))
                outp = ctx.enter_context(tc.tile_pool(name="gi1out", bufs=4))
                bias = ctx.enter_context(tc.tile_pool(name="gi1bias", bufs=1))
                bt_ = bias.tile([1, G3], BF16, name="wih1b", tag="wih1b")
                nc.sync.dma_start(bt_[:, :], wih1T[H:H + 1, :])
                # h0 staging shares the weight-residency slot (layer-0's
                # resident weights are dead here; layer-1's load comes after)
                h0sb = res_pool.tile([128, KT, T, B], BF16, name="h0sb",
                                     tag="wres")
                for t in range(T):
                    hwq(t).dma_start(h0sb[:, :, t, :], h0_tiles[t])
                for ch in range(NCH):
                    cs = slice(ch * CS, (ch + 1) * CS)
                    pts = [psum.tile([128, CS], F32, name="p1", tag="g1p")
                           for _ in range(MT)]
                    for ko in range(KT):
                        ks = slice(ko * 128, (ko + 1) * 128)
                        wt = stream.tile([128, G3], BF16, name="w1",
                                         tag="wst")
                        hwq(ko).dma_start(wt[:, :CS], wih1T[ks, cs])
                        for m in range(MT):
                            nc.tensor.matmul(
                                pts[m][:, :], h0sb[:, ko, 2 * m:2 * m + 2, :],
                                wt[:, :CS], start=(ko == 0), stop=False)
                    for m in range(MT):
                        nc.tensor.matmul(     # bias row via K=1 ones matmul
                            pts[m][:, :], ones_sb[:, :128], bt_[:, cs],
                            start=False, stop=True)
                        ot = outp.tile([128, CS], BF16, name="o1", tag="g1o")
                        nc.vector.tensor_copy(ot[:, :], pts[m][:, :])
                        nc.sync.dma_start(
                            gi_d[1][m * 128:(m + 1) * 128, cs], ot[:, :])

        def recurrence(layer, wres):
            """T GRU steps for one layer."""
            with ExitStack() as ctx:
                psum = ctx.enter_context(
                    tc.tile_pool(name="ghpsum", bufs=NCH, space="PSUM"))
                trps = ctx.enter_context(
                    tc.tile_pool(name="trpsum", bufs=2, space="PSUM"))
                hsb_p = ctx.enter_context(tc.tile_pool(name="hsb", bufs=2))
                gi_p = ctx.enter_context(tc.tile_pool(name="gis", bufs=2))
                ew = ctx.enter_context(tc.tile_pool(name="ew", bufs=1))
                hn_p = ctx.enter_context(tc.tile_pool(name="hnew", bufs=2))
                tr_p = ctx.enter_context(tc.tile_pool(name="htr", bufs=2))

                bh = bhh_pool.tile([1, G3], BF16, name="bh", tag="bh")
                nc.gpsimd.dma_start(bh[:, :], bhhn[layer][:, :])

                h_prev = None
                h_gathered = None   # DRAM AP of last AllGather output
                ag_tiles = []       # layer-0 AG outputs (for gi1)
                # chunk ch lives in PSUM tile ch//2, partition half ch%2 —
                # pairs of chunks share a bank and run in separate PE column
                # groups concurrently.
                nbank = (NCH + 1) // 2 if coltile else NCH

                def chunk_ap(pts, ch):
                    if coltile:
                        return pts[ch // 2][(ch % 2) * 64:(ch % 2) * 64 + 64,
                                            :]
                    return pts[ch][0:64, :]

                # interleave resident and streamed K-tiles so the streamed
                # DMA demand is spread evenly across the PE block
                NS = KT - R
                if R and NS:
                    ko_order = sorted(
                        range(KT),
                        key=lambda ko: ((ko - R + 0.5) / NS) if ko >= R
                        else ((ko + 0.5) / R))
                else:
                    ko_order = list(range(KT))
                nq = [0]   # stream-queue round robin counter

                for t in range(T):
                    # ---- gh = h_{t-1} @ W_hh^T (+ bhh_n) in PSUM ----
                    # bank-outer: each gate's bank fully accumulates before
                    # the next, so its gate math overlaps later banks.
                    pts = [psum.tile([128, CS] if coltile else [64, CS], F32,
                                     name="gh", tag="gh")
                           for _ in range(nbank)]
                    np_ = 128 if coltile else 64
                    if t > 0:
                        hsb = hsb_p.tile([128, KT, B], BF16, name="hsbt",
                                         tag="hsbt")
                        nc.gpsimd.dma_start(hsb[:, :, :], h_gathered)
                    for bk in range(nbank):
                        chl = [c for c in ([2 * bk, 2 * bk + 1] if coltile
                                           else [bk]) if c < NCH]
                        c0 = chl[0] * CS
                        cw = len(chl) * CS
                        # full-bank zero seed opens the accumulation group;
                        # every later matmul overlaps it (ordering + zeros
                        # with has_written set -> clean accumulate)
                        nc.tensor.matmul(pts[bk][:np_, :],
                                         ones_sb[:, :np_], zrow[:, :],
                                         start=True, stop=False)
                        if t > 0:
                            for ko in ko_order:
                                if ko < R:
                                    wt = wres[:, ko, c0:c0 + cw]
                                else:
                                    wtile = stream.tile(
                                        [128, 2 * CS], BF16, name="ws",
                                        tag="wsl", bufs=12)
                                    nq[0] += 1
                                    hwq(nq[0]).dma_start(
                                        wtile[:, :cw],
                                        whhT[layer][ko * 128:(ko + 1) * 128,
                                                    c0:c0 + cw])
                                    wt = wtile[:, :cw]
                                for i, ch in enumerate(chl):
                                    nc.tensor.matmul(
                                        chunk_ap(pts, ch), hsb[:, ko, :],
                                        wt[:, i * CS:(i + 1) * CS],
                                        start=False, stop=False)
                        for ch in chl:
                            nc.tensor.matmul(
                                chunk_ap(pts, ch), ones_sb[:, :64],
                                bh[:, ch * CS:(ch + 1) * CS],
                                start=False, stop=False)
                        nc.tensor.matmul(pts[bk][:np_, :],
                                         ones_sb[:, :np_], zrow[:, :],
                                         start=False, stop=True)

                    # ---- elementwise gates ----
                    gi_t = gi_p.tile([64, G3], BF16, name="git", tag="git")
                    nc.gpsimd.dma_start(
                        gi_t[:, :], gi_d[layer][t * 64:(t + 1) * 64, :])
                    s = ew.tile([64, 2 * PH], F32, name="s", tag="s")
                    rz = ew.tile([64, 2 * PH], F32, name="rz", tag="rz")
                    npre = ew.tile([64, PH], F32, name="npre", tag="npre")
                    nadd = ew.tile([64, PH], F32, name="nadd", tag="nadd")
                    nt = ew.tile([64, PH], F32, name="nt", tag="nt")
                    d = ew.tile([64, PH], F32, name="d", tag="d")
                    e = ew.tile([64, PH], F32, name="e", tag="e")
                    h_new = hn_p.tile([64, PH], F32, name="hnw", tag="hnw")
                    for ch in range(2 * NCHG):   # r and z chunks
                        cls = slice(ch * CS, (ch + 1) * CS)
                        nc.vector.tensor_add(s[:, cls], chunk_ap(pts, ch),
                                             gi_t[:, cls])
                    nc.scalar.activation(rz[:, :PH], s[:, :PH], ACT.Sigmoid)
                    nc.scalar.activation(rz[:, PH:], s[:, PH:], ACT.Sigmoid)
                    for j in range(NCHG):        # n chunks: r * gh_n
                        cls = slice(j * CS, (j + 1) * CS)
                        nc.vector.tensor_mul(npre[:, cls],
                                             chunk_ap(pts, 2 * NCHG + j),
                                             rz[:, cls])
                        nc.vector.tensor_add(nadd[:, cls], npre[:, cls],
                                             gi_t[:, 2 * PH + j * CS:
                                                  2 * PH + (j + 1) * CS])
                        nc.scalar.activation(nt[:, cls], nadd[:, cls],
                                             ACT.Tanh)
                        if t == 0:
                            nc.vector.tensor_mul(e[:, cls], rz[:, PH:][:, cls],
                                                 nt[:, cls])
                            nc.vector.tensor_sub(h_new[:, cls], nt[:, cls],
                                                 e[:, cls])
                        else:
                            nc.vector.tensor_sub(d[:, cls],
                                                 h_prev[:, cls],
                                                 nt[:, cls])
                            nc.vector.tensor_mul(e[:, cls], rz[:, PH:][:, cls],
                                                 d[:, cls])
                            nc.vector.tensor_add(h_new[:, cls], nt[:, cls],
                                                 e[:, cls])
                    h_prev = h_new

                    # ---- transpose h_new -> (PH,64): f32 out + bf16 AG ----
                    need_f32 = (layer == 1) or (t == T - 1)
                    hsh = tr_p.tile([128, len(cfg.tr), B], BF16, name="hsh",
                                    tag="hsh")
                    if need_f32:
                        htr = tr_p.tile([128, len(cfg.tr), B], F32,
                                        name="htf", tag="htf")
                    for j, (off, sz) in enumerate(cfg.tr):
                        tp = trps.tile([128, 64], F32, name="tp", tag="tp")
                        nc.tensor.transpose(tp[:sz, :],
                                            h_new[:, off:off + sz],
                                            ident[:, :])
                        if need_f32:
                            nc.vector.tensor_copy(htr[:sz, j, :], tp[:sz, :])
                        nc.vector.tensor_copy(hsh[:sz, j, :], tp[:sz, :])

                    if layer == 1:
                        for j, (off, sz) in enumerate(cfg.tr):
                            nc.sync.dma_start(out1[t, off:off + sz, :],
                                              htr[:sz, j, :])
                    if layer == 0 and t == T - 1:
                        for j, (off, sz) in enumerate(cfg.tr):
                            nc.sync.dma_start(hid0[off:off + sz, :],
                                              htr[:sz, j, :])

                    # ---- AllGather h across cores ----
                    if layer == 1 and t == T - 1:
                        continue
                    # local shard (PH, B) in pl-major local-unit order; the
                    # 8 shards concatenate to the packed (128, KT, B) layout.
                    hshard = dram2.tile([PH, B], BF16, name="hshd",
                                        tag="hshd")
                    for j, (off, sz) in enumerate(cfg.tr):
                        nc.gpsimd.dma_start(hshard[off:off + sz, :],
                                            hsh[:sz, j, :])
                    hc = dram2.tile([128, KT, B], BF16, name="hc", tag="hc",
                                    bufs=max(T, 2), addr_space="Shared")
                    nc.gpsimd.collective_compute(
                        "AllGather", mybir.AluOpType.bypass,
                        replica_groups=rg,
                        ins=[hshard[:, :].opt()],
                        outs=[hc[:, :, :].opt()],
                    )
                    if layer == 0:
                        ag_tiles.append(hc[:, :, :])
                    h_gathered = hc[:, :, :]
                return ag_tiles

        # ---- phase structure ----
        gi0_gemm()
        wres0 = load_resident(0)
        h0_tiles = recurrence(0, wres0)
        gi1_gemm(h0_tiles)
        wres1 = load_resident(1)
        recurrence(1, wres1)

    nc.compile()
    return nc


# --------------------------------------------------------------------------
# host side
# --------------------------------------------------------------------------

def _unit_order(cfg: Cfg, c: int) -> np.ndarray:
    """Global hidden-unit index for core c's local units 0..PH-1.
    Local order is pl-major: j = pl*KT + ko -> unit ko*128 + c*PL + pl."""
    ar = np.arange(cfg.PH)
    return (ar % cfg.KT) * 128 + c * PL + (ar // cfg.KT)


def prep_inputs(cfg: Cfg, inputs, W_ih0, W_hh0, b_ih0, b_hh0,
                W_ih1, W_hh1, b_ih1, b_hh1):
    B, T, H, PH, IN0 = cfg.B, cfg.T, cfg.H, cfg.PH, cfg.IN0
    inputs = np.asarray(inputs, np.float32)
    assert inputs.shape == (B, T, IN0)
    xT = np.empty((IN0 + 1, cfg.BT), dtype=np.float32)
    xT[:IN0] = inputs.transpose(2, 1, 0).reshape(IN0, T * B)  # col = t*B+b
    xT[IN0] = 1.0
    xT = xT.astype(BF16NP)

    arrs = {k: np.asarray(v, np.float32) for k, v in dict(
        W_ih0=W_ih0, W_hh0=W_hh0, b_ih0=b_ih0, b_hh0=b_hh0,
        W_ih1=W_ih1, W_hh1=W_hh1, b_ih1=b_ih1, b_hh1=b_hh1).items()}

    def shard(c):
        unit = _unit_order(cfg, c)
        idx = np.concatenate([g * H + unit for g in range(3)])

        def wt(W, b_i, b_h, kdim):
            out = np.empty((kdim + 1, 3 * PH), dtype=np.float32)
            out[:kdim] = W[idx].T
            bias = b_i[idx].copy()
            bias[:2 * PH] += b_h[idx][:2 * PH]   # r,z: b_ih+b_hh; n: b_ih
            out[kdim] = bias
            return out.astype(BF16NP)

        m = {
            "xT": xT,
            "wih0T": wt(arrs["W_ih0"], arrs["b_ih0"], arrs["b_hh0"], IN0),
            "wih1T": wt(arrs["W_ih1"], arrs["b_ih1"], arrs["b_hh1"], H),
            "whh0T": np.ascontiguousarray(arrs["W_hh0"][idx].T).astype(BF16NP),
            "whh1T": np.ascontiguousarray(arrs["W_hh1"][idx].T).astype(BF16NP),
        }
        for l in range(2):
            row = np.zeros((1, 3 * PH), dtype=np.float32)
            row[0, 2 * PH:] = arrs[f"b_hh{l}"][idx][2 * PH:]
            m[f"bhhn{l}"] = row.astype(BF16NP)
        return m

    return [shard(c) for c in range(NCORES)]


def assemble(cfg: Cfg, outs):
    T, B, H, KT = cfg.T, cfg.B, cfg.H, cfg.KT
    # local unit j=(pl*KT+ko) on core c -> global unit ko*128 + c*PL + pl
    out1 = np.stack([outs[c]["out1"] for c in range(NCORES)])  # (8,T,PH,B)
    out1 = out1.reshape(NCORES, T, PL, KT, B)
    output = np.ascontiguousarray(
        out1.transpose(1, 4, 3, 0, 2).reshape(T, B, H))
    hid0 = np.stack([outs[c]["hid0"] for c in range(NCORES)])  # (8,PH,B)
    hid0 = hid0.reshape(NCORES, PL, KT, B)
    h0_last = hid0.transpose(3, 2, 0, 1).reshape(B, H)
    hidden = np.stack([h0_last, output[T - 1]], axis=0)
    return output.astype(np.float32), hidden.astype(np.float32)


LAST_RESULTS = None


def _ensure_ntff_hook():
    """The agent image's antenv lacks axon_hooks; recreate it so
    trace=True can drive NTFF profiling via the injected libaxon so."""
    try:
        from antenv.axon_hooks import get_axon_ntff_profile_hook  # noqa: F401
        return
    except ImportError:
        pass
    import sys
    import types

    import antenv

    mod = types.ModuleType("antenv.axon_hooks")
    _hook = [None]
    mod.set_axon_ntff_profile_hook = lambda h: _hook.__setitem__(0, h)
    mod.get_axon_ntff_profile_hook = lambda: _hook[0]
    sys.modules["antenv.axon_hooks"] = mod
    antenv.axon_hooks = mod
    try:
        if "/root/.axon_site" not in sys.path:
            sys.path.insert(0, "/root/.axon_site")
        from trn_agent_boot.trn_boot import _ntff_profile_via_ctypes
        so = "/opt/axon/libaxon_pjrt.so"
        if os.path.exists(so):
            mod.set_axon_ntff_profile_hook(_ntff_profile_via_ctypes(so))
    except Exception:
        pass


def kernel(inputs, W_ih0, W_hh0, b_ih0, b_hh0, W_ih1, W_hh1, b_ih1, b_hh1):
    global LAST_RESULTS
    cfg = Cfg(R=int(os.environ.get("GRU_R", "20")),
              stream_bufs=int(os.environ.get("GRU_SB", "4")))
    nc = build_nc(cfg)
    in_maps = prep_inputs(cfg, inputs, W_ih0, W_hh0, b_ih0, b_hh0,
                          W_ih1, W_hh1, b_ih1, b_hh1)
    trace = bool(int(os.environ.get("GRU_TRACE", "0")))
    if trace:
        _ensure_ntff_hook()
    res = run_bass_kernel_spmd(nc, in_maps, list(range(NCORES)), trace=trace)
    LAST_RESULTS = res
    return assemble(cfg, res.results)
